# revision 21
# baseline (speedup 1.0000x reference)
"""Trainium2 Bass kernel for nn_DQN_9904194584789 (GNN message passing DQN).

Reference math (B=16, A=256, U=64, T=3):
    cur_sol = x[:,0,:]; mask = x[:,1,:]; w = x[:,2:,:]          # [B,A,A]
    adj = (w != 0)
    e1 = cur_sol[:,:,None] @ W0                                  # rank-1
    e3 = (sum_j relu(w[...,None] * W3) / A) @ W2                 # collapses:
         sum_j relu(w*c) = P*relu(c) + N*relu(-c) elementwise in c, with
         P = sum_j relu(w), N = sum_j relu(-w) = -M, M = sum_j min(w, 0)
         => e3 = P x (relu(W3)@W2/A) + M x (-relu(-W3)@W2/A) = P x Fp + M x Fn
    base = e1 + e3 (loop invariant);  emb_1 = relu(base)
    emb_{t+1} = relu(base + c_t) with c_t = W1^T @ colsum(emb_t)/A   (fast
    path: w has no exact zeros so adj is all-ones and adj@emb collapses to
    a colsum broadcast).
    heads: dueling MLP on emb_3; out = pa + [sum(psv) - sum(pa) + kc] + 10*mask
    with kc = A*v_b2 - (A-1)*a_b3.

Sharding: pure data-parallel over batch B (2 batches per core x 8 cores),
replicated weights host-packed into one [128, NW] buffer (fp16 payloads as
raw bits in f32 columns, incl. a host-packed fp16 identity).  Host work is
only slicing / layout packing / dtype casts (x cast to fp16); all model math
runs on device.

Fast-path structure (both batches fused into [., 512] ops):
  - P/M row-sum stats via tensor_scalar relu/min chunk ops with accum_out
    (DVE 5 chunks + Pool 3 chunks in parallel), cast once to fp16, PE
    transposes -> G [2, 512]
  - base = Fts^T @ G + W0row^T @ cur_row (two matmuls into one PSUM bank)
  - 3 relu iterations on DVE reading base straight from PSUM, with colsum
    accum_out; c = W1^T cs/A via fp16 hi+lo matmuls
  - dueling heads: row-sum corrections come free from accum_out on the
    h2/hv relu legs + two tiny matmuls (no big reduces); 10*mask is a
    rank-1 fp16 matmul accumulated into the pa PSUM bank; kc folds into K.

General path (any exact zero in w): the original f32 kernel with real
adjacency matmuls, unchanged.
"""

import numpy as np
from contextlib import ExitStack

import concourse.bass as bass
import concourse.bacc as bacc
import concourse.tile as tile
from concourse import mybir
from concourse.bass_utils import run_bass_kernel_spmd
from concourse.masks import make_identity

f32 = mybir.dt.float32
f16 = mybir.dt.float16
Alu = mybir.AluOpType
Act = mybir.ActivationFunctionType
AxX = mybir.AxisListType.X

B, A, U, HID = 16, 256, 64, 64
NCORES = 8
BPC = B // NCORES  # batches per core
INV_A = 1.0 / A

WEIGHT_NAMES = [
    "W0", "W1", "W2", "W3",
    "a_w1", "a_b1", "a_w2", "a_b2", "a_w3", "a_b3",
    "v_w1", "v_b1", "v_w2", "v_b2",
]

# ---------------------------------------------------------------------------
# fast-path wpk layout: [128, NW] f32; fp16 payloads packed pairwise as bits.
_c = 0
def _adv(n):
    global _c
    r = _c
    _c += n
    return r

CW1H = _adv(32)     # (W1/A) fp16 hi      [64, 64]
CW1L = _adv(32)     # (W1/A) fp16 lo      [64, 64]
CW2H = _adv(32)     # W2 fp16             [64, 64]
CW3H = _adv(1)      # W3 col fp16         [64, 1]
CAW1 = _adv(32)     # a_w1 fp16           [64, 64]
CAW2 = _adv(16)     # a_w2 fp16           [64, 32]
CVW1 = _adv(32)     # v_w1 fp16           [64, 64]
CW0R = _adv(32)     # W0 row fp16         [1, 64] (partition 0)
CA3M = _adv(1)      # [a_w3; 10.0] fp16   [33, 1]
CVW2H = _adv(1)     # v_w2 fp16 col       [64, 1]
CA3NH = _adv(1)     # -a_w3 fp16 col      [32, 1]
CAB1 = _adv(1)      # a_b1 f32 col        [64, 1]
CVB1 = _adv(1)      # v_b1 f32 col        [64, 1]
CAB2 = _adv(1)      # a_b2 f32 col        [32, 1]
CAB3 = _adv(1)      # a_b3 scalar         [1, 1]
CVB2 = _adv(1)      # v_b2 scalar         [1, 1]
CIDT = _adv(64)     # identity fp16       [128, 128]
NW = _c


def _pack_fast_weights(inputs: dict) -> np.ndarray:
    wp = np.zeros((128, NW), dtype=np.float32)

    def place(col, arr):  # fp16 bits packed pairwise into f32 columns
        raw = np.ascontiguousarray(
            np.asarray(arr, np.float32).astype(np.float16)
        ).view(np.uint16)
        k = raw.shape[1]
        pad = np.zeros((raw.shape[0], (k + 1) // 2 * 2), np.uint16)
        pad[:, :k] = raw
        fview = pad.view(np.float32)
        wp[:fview.shape[0], col:col + fview.shape[1]] = fview

    # 1/A = 1/256 is a power of two: W1/A is an exact f32 re-encoding;
    # hi+lo fp16 split keeps the W1 product near-f32 accurate.
    w1a = np.asarray(inputs["W1"], np.float32) * INV_A
    w1ah = w1a.astype(np.float16).astype(np.float32)
    place(CW1H, w1a)
    place(CW1L, w1a - w1ah)
    place(CW2H, inputs["W2"])
    place(CW3H, np.asarray(inputs["W3"], np.float32).T)   # [64, 1]
    place(CAW1, inputs["a_w1"])
    place(CAW2, inputs["a_w2"])
    place(CVW1, inputs["v_w1"])
    place(CW0R, inputs["W0"])                              # [1, 64] row
    a3m = np.zeros((33, 1), np.float32)
    a3m[:32, 0] = np.asarray(inputs["a_w3"], np.float32)[:, 0]
    a3m[32, 0] = 10.0
    place(CA3M, a3m)
    place(CVW2H, np.asarray(inputs["v_w2"], np.float32))
    place(CA3NH, -np.asarray(inputs["a_w3"], np.float32))
    place(CIDT, np.eye(128, dtype=np.float32))
    wp[:64, CAB1] = inputs["a_b1"]
    wp[:64, CVB1] = inputs["v_b1"]
    wp[:32, CAB2] = inputs["a_b2"]
    wp[0, CAB3] = inputs["a_b3"][0]
    wp[0, CVB2] = inputs["v_b2"][0]
    return wp


def _build_fast() -> bass.Bass:
    # Bacc (not raw Bass): its finalize() runs move_matmul_waits_to_ldweights
    # + generate_event_semaphores for the TRN2 one-wait-per-inst constraint.
    nc = bacc.Bacc(
        "TRN2", target_bir_lowering=False, debug=False, num_devices=NCORES
    )
    w16d = nc.declare_dram_parameter("w16", [BPC, A, A], f16, isOutput=False)
    auxc = nc.declare_dram_parameter("aux_c", [1, BPC * A], f16, isOutput=False)
    auxm = nc.declare_dram_parameter("aux_m", [1, BPC * A], f16, isOutput=False)
    wpd = nc.declare_dram_parameter("wpk", [128, NW], f32, isOutput=False)
    outd = nc.declare_dram_parameter("out", [1, BPC * A], f32, isOutput=True)

    NBA = BPC * A  # 512

    with tile.TileContext(nc) as tc, ExitStack() as ctx, \
         nc.allow_low_precision("fp16 colsum accums; tol 5e-3"):
        cp = ctx.enter_context(tc.tile_pool(name="const", bufs=1))
        sp = ctx.enter_context(tc.tile_pool(name="scratch", bufs=2))
        dv = ctx.enter_context(tc.tile_pool(name="dumpv", bufs=2))
        dp = ctx.enter_context(tc.tile_pool(name="dumpp", bufs=2))

        # ---------- input DMAs, issued first on distinct queues -----------
        # batch1 feeds the (slower) ACT chain -> land it first on sync.
        wt4 = cp.tile([128, 2 * BPC, A], f16, tag="wt4")
        nc.sync.dma_start(
            wt4[:, 2:4, :],
            w16d[1, :, :].rearrange("(t p) j -> p t j", p=128),
        )
        wp = cp.tile([128, NW], f32, tag="wp")
        nc.scalar.dma_start(wp[:], wpd[:])
        nc.scalar.dma_start(
            wt4[:, 0:2, :],
            w16d[0, :, :].rearrange("(t p) j -> p t j", p=128),
        )
        cur = cp.tile([1, NBA], f16, tag="cur")
        nc.gpsimd.dma_start(cur[:], auxc[:])
        # per-batch h2 tiles [33, 256]: partition 32 carries the mask row so
        # the pa matmul computes out_a + 10*mask in one [33,1]^T @ [33,256].
        h2A = cp.tile([HID // 2 + 1, A], f16, tag="h2A")
        nc.gpsimd.dma_start(h2A[32:33, :], auxm[:, 0:A])
        h2B = cp.tile([HID // 2 + 1, A], f16, tag="h2B")
        nc.gpsimd.dma_start(h2B[32:33, :], auxm[:, A:2 * A])

        # views of host-packed weights
        w1hh = wp[0:64, CW1H:CW1H + 32].bitcast(f16)          # [64, 64]
        w1lh = wp[0:64, CW1L:CW1L + 32].bitcast(f16)          # [64, 64]
        w2h = wp[0:64, CW2H:CW2H + 32].bitcast(f16)           # [64, 64]
        w3h = wp[0:64, CW3H:CW3H + 1].bitcast(f16)[:, 0:1]    # [64, 1]
        aw1h = wp[0:64, CAW1:CAW1 + 32].bitcast(f16)          # [64, 64]
        aw2h = wp[0:64, CAW2:CAW2 + 16].bitcast(f16)          # [64, 32]
        vw1h = wp[0:64, CVW1:CVW1 + 32].bitcast(f16)          # [64, 64]
        w0r = wp[0:1, CW0R:CW0R + 32].bitcast(f16)            # [1, 64]
        a3m = wp[0:33, CA3M:CA3M + 1].bitcast(f16)[:, 0:1]    # [33, 1]
        vw2h = wp[0:64, CVW2H:CVW2H + 1].bitcast(f16)[:, 0:1]
        a3nh = wp[0:32, CA3NH:CA3NH + 1].bitcast(f16)[:, 0:1]
        ident = wp[:, CIDT:CIDT + 64].bitcast(f16)            # [128, 128]
        ab1f = wp[0:64, CAB1:CAB1 + 1]
        vb1f = wp[0:64, CVB1:CVB1 + 1]
        ab2f = wp[0:32, CAB2:CAB2 + 1]

        with tc.tile_pool(name="pmm", bufs=2, space="PSUM") as pmm, \
             tc.tile_pool(name="pbase", bufs=1, space="PSUM") as pbase, \
             tc.tile_pool(name="phead", bufs=1, space="PSUM") as phead:

            # ---------- setup (runs in the input-DMA shadow) --------------
            # e3 = T x (|W3|@W2/2A) + S x (W3@W2/2A) with T = sum|w|,
            # S = sum w  (from e3 = P*Fp + M*Fn, P=(S+T)/2, M=(S-T)/2).
            w3p = cp.tile([U, 1], f16, tag="w3p")
            nc.scalar.activation(w3p[:], w3h, Act.Abs, scale=0.5 * INV_A)
            w3i = cp.tile([U, 1], f16, tag="w3i")
            nc.scalar.activation(w3i[:], w3h, Act.Identity, scale=0.5 * INV_A)
            psF2 = pmm.tile([U, 2], f32, tag="tp")
            nc.tensor.matmul(psF2[:, 0:1], w2h, w3p[:])
            nc.tensor.matmul(psF2[:, 1:2], w2h, w3i[:])
            Fc16 = cp.tile([U, 2], f16, tag="Fc16")
            nc.scalar.activation(Fc16[:], psF2[:], Act.Identity)
            psFT = pmm.tile([2, U], f16, tag="tp")
            nc.tensor.transpose(psFT[:], Fc16[:], ident[0:U, 0:U])
            Fts = cp.tile([2, U], f16, tag="Fts")
            nc.scalar.activation(Fts[:], psFT[:], Act.Identity)

            # kc = A*v_b2 - (A-1)*a_b3 (scalar part of the dueling combine)
            t256 = cp.tile([1, 1], f32, tag="t256")
            nc.gpsimd.tensor_scalar(
                t256[:], wp[0:1, CVB2:CVB2 + 1], float(A), None, Alu.mult
            )
            kc = cp.tile([1, 1], f32, tag="kc")
            nc.gpsimd.tensor_scalar(
                kc[:], wp[0:1, CAB3:CAB3 + 1], -float(A - 1), t256[:],
                Alu.mult, Alu.add,
            )
            zz = cp.tile([U, A], f16, tag="zz")
            nc.gpsimd.memset(zz[:], 0.0)

            # ---------- T/S stats -> G -> base, per batch (B first) -------
            # batch1's data lands first and feeds the slower ACT chain.
            TScB = cp.tile([128, 4], f16, tag="TScB")
            with nc.allow_low_precision("fp16 row-sum stats; tol 5e-3"):
                nc.vector.tensor_reduce(
                    TScB[:, 0:4:2], wt4[:, 2:4, :], axis=AxX,
                    op=Alu.add, apply_absolute_value=True,
                )
                nc.vector.tensor_reduce(
                    TScB[:, 1:4:2], wt4[:, 2:4, :], axis=AxX,
                    op=Alu.add
                )
            GB = cp.tile([2, 2, 128], f16, tag="GB")
            psT = pmm.tile([2, 2, 128], f16, tag="tp")
            nc.tensor.transpose(psT[:, 0, :], TScB[:, 0:2], ident[:])
            nc.tensor.transpose(psT[:, 1, :], TScB[:, 2:4], ident[:])
            nc.vector.tensor_copy(GB[:], psT[:])
            ps_bB = pbase.tile([U, A], f32, tag="psbB")
            nc.tensor.matmul(ps_bB[:], Fts[:],
                             GB[:].rearrange("r c p -> r (c p)"),
                             start=True, stop=False)
            nc.tensor.matmul(ps_bB[:], w0r, cur[:, A:2 * A], start=False,
                             stop=True)

            TScA = cp.tile([128, 4], f16, tag="TScA")
            with nc.allow_low_precision("fp16 row-sum stats; tol 5e-3"):
                nc.vector.tensor_reduce(
                    TScA[:, 0:4:2], wt4[:, 0:2, :], axis=AxX,
                    op=Alu.add, apply_absolute_value=True,
                )
                nc.vector.tensor_reduce(
                    TScA[:, 1:4:2], wt4[:, 0:2, :], axis=AxX,
                    op=Alu.add
                )
            GA = cp.tile([2, 2, 128], f16, tag="GA")
            psT = pmm.tile([2, 2, 128], f16, tag="tp")
            nc.tensor.transpose(psT[:, 0, :], TScA[:, 0:2], ident[:])
            nc.tensor.transpose(psT[:, 1, :], TScA[:, 2:4], ident[:])
            nc.vector.tensor_copy(GA[:], psT[:])
            ps_bA = pbase.tile([U, A], f32, tag="psbA")
            nc.tensor.matmul(ps_bA[:], Fts[:],
                             GA[:].rearrange("r c p -> r (c p)"),
                             start=True, stop=False)
            nc.tensor.matmul(ps_bA[:], w0r, cur[:, 0:A], start=False,
                             stop=True)

            # ---------- message passing ----------------------------------
            # independent per-batch chains: batch0 on DVE, batch1 on ACT;
            # each chain is engine-self-contained (PSUM banks and scratch
            # tiles are single-engine to avoid serializing hazards).
            cs_a = sp.tile([U, 1], f16, tag="cs_a")
            cs_b = sp.tile([U, 1], f16, tag="cs_b")
            dm = dp.tile([U, A], f16, tag="dms")
            nc.scalar.activation(dm[:], ps_bB[:], Act.Relu,
                                 accum_out=cs_b[:])
            dm = dv.tile([U, A], f16, tag="dme")
            nc.vector.tensor_scalar(
                dm[:], ps_bA[:], 0.0, None, Alu.max,
                op1=Alu.add, accum_out=cs_a[:],
            )
            emb_a = cp.tile([U, A], f16, tag="emb_a")
            emb_b = cp.tile([U, A], f16, tag="emb_b")
            for it in range(2):
                psCB = pmm.tile([U, 1], f32, tag="tp")
                nc.tensor.matmul(psCB[:], w1hh, cs_b[:], start=True,
                                 stop=False)
                nc.tensor.matmul(psCB[:], w1lh, cs_b[:], start=False,
                                 stop=True)
                csbB = sp.tile([U, 1], f32, tag="csbB")
                nc.scalar.activation(csbB[:], psCB[:], Act.Identity)
                psCA = pmm.tile([U, 1], f32, tag="tp")
                nc.tensor.matmul(psCA[:], w1hh, cs_a[:], start=True,
                                 stop=False)
                nc.tensor.matmul(psCA[:], w1lh, cs_a[:], start=False,
                                 stop=True)
                csbA = sp.tile([U, 1], f32, tag="csbA")
                nc.vector.tensor_copy(csbA[:], psCA[:])
                if it == 0:
                    cs_a = sp.tile([U, 1], f16, tag="cs_a2")
                    cs_b = sp.tile([U, 1], f16, tag="cs_b2")
                    dm = dp.tile([U, A], f16, tag="dms")
                    nc.scalar.activation(dm[:], ps_bB[:], Act.Relu,
                                         bias=csbB[:], accum_out=cs_b[:])
                    dm = dv.tile([U, A], f16, tag="dme")
                    nc.vector.scalar_tensor_tensor(
                        dm[:], ps_bA[:], csbA[:], zz[:],
                        Alu.add, Alu.max, accum_out=cs_a[:],
                    )
                else:
                    nc.scalar.activation(emb_b[:], ps_bB[:],
                                         Act.Relu, bias=csbB[:])
                    nc.vector.tensor_scalar(
                        emb_a[:], ps_bA[:], csbA[:], 0.0,
                        Alu.add, op1=Alu.max,
                    )

            # ---------- dueling heads: per-batch chains (B first) ---------
            ph1B = phead.tile([HID, A], f32, tag="hb")
            nc.tensor.matmul(ph1B[:], aw1h, emb_b[:])
            phvB = phead.tile([HID, A], f32, tag="vb")
            nc.tensor.matmul(phvB[:], vw1h, emb_b[:])
            h1B = cp.tile([HID, A], f16, tag="h1B")
            nc.scalar.activation(h1B[:], ph1B[:], Act.Relu, bias=ab1f)

            ph1A = phead.tile([HID, A], f32, tag="ha")
            nc.tensor.matmul(ph1A[:], aw1h, emb_a[:])
            phvA = phead.tile([HID, A], f32, tag="va")
            nc.tensor.matmul(phvA[:], vw1h, emb_a[:])
            h1A = cp.tile([HID, A], f16, tag="h1A")
            nc.vector.tensor_scalar(
                h1A[:], ph1A[:], ab1f, 0.0, Alu.add, op1=Alu.max,
            )

            ph2B = phead.tile([HID // 2, A], f32, tag="hb")
            nc.tensor.matmul(ph2B[:], aw2h, h1B[:])
            hvcsb = sp.tile([U, 1], f16, tag="hvcsb")
            dmv = dp.tile([U, A], f16, tag="dms")
            nc.scalar.activation(dmv[:], phvB[:], Act.Relu, bias=vb1f,
                                 accum_out=hvcsb[:])
            h2csb = sp.tile([HID // 2, 1], f16, tag="h2csb")
            nc.scalar.activation(h2B[0:32, :], ph2B[:], Act.Relu,
                                 bias=ab2f, accum_out=h2csb[:])

            ph2A = phead.tile([HID // 2, A], f32, tag="ha")
            nc.tensor.matmul(ph2A[:], aw2h, h1A[:])
            hvcsa = sp.tile([U, 1], f16, tag="hvcsa")
            dmv = dv.tile([U, A], f16, tag="dme")
            nc.vector.scalar_tensor_tensor(
                dmv[:], phvA[:], vb1f, zz[:], Alu.add, Alu.max,
                accum_out=hvcsa[:],
            )
            h2csa = sp.tile([HID // 2, 1], f16, tag="h2csa")
            nc.vector.scalar_tensor_tensor(
                h2A[0:32, :], ph2A[:], ab2f, zz[0:32, :],
                Alu.add, Alu.max, accum_out=h2csa[:],
            )

            # K columns: fp16 casts of the accums feed tiny PE matmuls
            psK1 = pmm.tile([1, 1], f32, tag="tp")
            nc.tensor.matmul(psK1[:], vw2h, hvcsb[:], start=True, stop=False)
            nc.tensor.matmul(psK1[:], a3nh, h2csb[:], start=False, stop=True)
            paB = phead.tile([1, A], f32, tag="hb")
            nc.tensor.matmul(paB[:], a3m, h2B[:])
            K1 = sp.tile([1, 1], f32, tag="K1")
            nc.vector.tensor_scalar(K1[:], psK1[:], kc[0:1, 0:1], None,
                                    Alu.add)
            FINb = cp.tile([1, A], f32, tag="FINb")
            nc.scalar.activation(FINb[:], paB[:], Act.Identity,
                                 bias=K1[0:1, 0:1])
            nc.scalar.dma_start(outd[:, A:2 * A], FINb[:])

            psK0 = pmm.tile([1, 1], f32, tag="tp")
            nc.tensor.matmul(psK0[:], vw2h, hvcsa[:], start=True, stop=False)
            nc.tensor.matmul(psK0[:], a3nh, h2csa[:], start=False, stop=True)
            paA = phead.tile([1, A], f32, tag="ha")
            nc.tensor.matmul(paA[:], a3m, h2A[:])
            K0 = sp.tile([1, 1], f32, tag="K0")
            nc.vector.tensor_scalar(K0[:], psK0[:], kc[0:1, 0:1], None,
                                    Alu.add)
            FINa = cp.tile([1, A], f32, tag="FINa")
            nc.vector.tensor_scalar(
                FINa[:], paA[:], K0[0:1, 0:1], None, Alu.add
            )
            nc.sync.dma_start(outd[:, 0:A], FINa[:])

    return nc


# ---------------------------------------------------------------------------
# General path (exact zeros in w): original f32 kernel, unchanged.

WP_W1 = 0
WP_W2 = 64
WP_W3 = 128
WP_AB1 = 129
WP_VB1 = 130
WP_AB2 = 131
WP_VW2 = 132
WP_AB3 = 133
WP_VB2 = 134
WP_AW1H = 135
WP_AW2H = 167
WP_AW3H = 183
WP_VW1H = 184
WP_W0C = 216
WP_W1HH = 217
WP_W1LH = 249
WP_AW3F = 280
WP_W2H = 281
NWP = 313


def _pack_weights(inputs: dict) -> np.ndarray:
    wp = np.zeros((64, NWP), dtype=np.float32)
    wp[:, WP_W1:WP_W1 + 64] = inputs["W1"]
    wp[:, WP_W2:WP_W2 + 64] = inputs["W2"]
    wp[:, WP_W3] = inputs["W3"][0]
    wp[:, WP_AB1] = inputs["a_b1"]
    wp[:, WP_VB1] = inputs["v_b1"]
    wp[:32, WP_AB2] = inputs["a_b2"]
    wp[:, WP_VW2] = inputs["v_w2"][:, 0]
    wp[0, WP_AB3] = inputs["a_b3"][0]
    wp[0, WP_VB2] = inputs["v_b2"][0]
    wp[:32, WP_AW3F] = inputs["a_w3"][:, 0]

    def place(col, arr):
        raw = np.ascontiguousarray(
            np.asarray(arr, np.float32).astype(np.float16)
        ).view(np.uint16)
        k = raw.shape[1]
        pad = np.zeros((raw.shape[0], (k + 1) // 2 * 2), np.uint16)
        pad[:, :k] = raw
        fview = pad.view(np.float32)
        wp[:fview.shape[0], col:col + fview.shape[1]] = fview

    place(WP_AW1H, inputs["a_w1"])
    place(WP_AW2H, inputs["a_w2"])
    place(WP_AW3H, inputs["a_w3"][:, 0:1])
    place(WP_VW1H, inputs["v_w1"])
    place(WP_W0C, inputs["W0"].T)
    place(WP_W2H, inputs["W2"])
    w1 = np.asarray(inputs["W1"], np.float32)
    w1h = w1.astype(np.float16).astype(np.float32)
    place(WP_W1HH, w1)
    place(WP_W1LH, w1 - w1h)
    return wp


def _build_general() -> bass.Bass:
    nc = bacc.Bacc(
        "TRN2", target_bir_lowering=False, debug=False, num_devices=NCORES
    )
    xs = nc.declare_dram_parameter("xs", [BPC, A + 2, A], f32, isOutput=False)
    wpd = nc.declare_dram_parameter("wpack", [64, NWP], f32, isOutput=False)
    out = nc.declare_dram_parameter("out", [BPC, A], f32, isOutput=True)

    with tile.TileContext(nc) as tc, ExitStack() as ctx, \
         nc.allow_low_precision("fp16 colsum accums; tol 5e-3"):
        cp = ctx.enter_context(tc.tile_pool(name="const", bufs=1))
        sp = ctx.enter_context(tc.tile_pool(name="scratch", bufs=2))

        wp = cp.tile([64, NWP], f32, tag="wp")
        nc.sync.dma_start(wp[:], wpd[:])
        wt4 = cp.tile([128, 2 * BPC, A], f32, tag="wt4")
        for b in range(BPC):
            nc.scalar.dma_start(
                wt4[:, 2 * b: 2 * b + 2, :],
                xs[b, 2: A + 2, :].rearrange("(t p) j -> p t j", p=128),
            )
        csc = cp.tile([128, 2 * BPC], f32, tag="csc")
        for b in range(BPC):
            nc.gpsimd.dma_start(
                csc[:, 2 * b: 2 * b + 2],
                xs[b, 0, :].rearrange("(t p) -> p t", p=128),
            )
        mrow = cp.tile([1, BPC * A], f32, tag="mrow")
        nc.gpsimd.dma_start(
            mrow[:].rearrange("p (b a) -> p b a", b=BPC),
            xs[:, 1, :][None, :, :],
        )

        aw1h = wp[:, WP_AW1H:WP_AW1H + 32].bitcast(f16)
        aw2h = wp[:, WP_AW2H:WP_AW2H + 16].bitcast(f16)
        aw3h = wp[0:32, WP_AW3H:WP_AW3H + 1].bitcast(f16)[:, 0:1]
        vw1h = wp[:, WP_VW1H:WP_VW1H + 32].bitcast(f16)
        w0c = wp[:, WP_W0C:WP_W0C + 1].bitcast(f16)[:, 0:1]

        ident = cp.tile([128, 128], f16, tag="ident")
        make_identity(nc, ident[:])
        identf = cp.tile([128, 128], f32, tag="identf")
        make_identity(nc, identf[:])

        with tc.tile_pool(name="psetup", bufs=2, space="PSUM") as psetup:
            w2h = wp[:, WP_W2H:WP_W2H + 32].bitcast(f16)
            w3p = cp.tile([U, 1], f16, tag="w3p")
            nc.scalar.activation(w3p[:], wp[:, WP_W3:WP_W3 + 1], Act.Relu)
            w3n = cp.tile([U, 1], f16, tag="w3n")
            nc.scalar.activation(w3n[:], wp[:, WP_W3:WP_W3 + 1], Act.Relu,
                                 scale=-1.0)
            Fc = cp.tile([U, 3], f16, tag="Fc")
            nc.vector.tensor_copy(Fc[:, 0:1], w0c)
            pspc = psetup.tile([U, 1], f32, tag="pscol")
            nc.tensor.matmul(pspc[:], w2h, w3p[:])
            nc.scalar.mul(Fc[:, 1:2], pspc[:], INV_A)
            psnc = psetup.tile([U, 1], f32, tag="pscol")
            nc.tensor.matmul(psnc[:], w2h, w3n[:])
            nc.scalar.mul(Fc[:, 2:3], psnc[:], INV_A)
            psF = psetup.tile([3, U], f16, tag="psF")
            nc.tensor.transpose(psF[:], Fc[:], ident[0:U, 0:U])
            F = cp.tile([3, U], f16, tag="F")
            nc.vector.tensor_copy(F[:], psF[:])

        t256 = cp.tile([1, 1], f32, tag="t256")
        nc.gpsimd.tensor_scalar(
            t256[:], wp[0:1, WP_VB2:WP_VB2 + 1], float(A), None, Alu.mult
        )
        kc = cp.tile([1, 1], f32, tag="kc")
        nc.gpsimd.tensor_scalar(
            kc[:], wp[0:1, WP_AB3:WP_AB3 + 1], -float(A - 1), t256[:],
            Alu.mult, Alu.add,
        )

        m10 = cp.tile([1, BPC * A], f32, tag="m10")
        nc.scalar.mul(m10[:], mrow[:], 10.0)

        FIN = cp.tile([1, BPC * A], f32, tag="FIN")

        with tc.tile_pool(name="pmm", bufs=1, space="PSUM") as pmm, \
             tc.tile_pool(name="pbase", bufs=2, space="PSUM") as pbase, \
             tc.tile_pool(name="phead", bufs=2, space="PSUM") as phead:
            for b in range(BPC):
                Tb = sp.tile([128, 2], f32, tag="Tb")
                nc.vector.tensor_reduce(
                    Tb[:], wt4[:, 2 * b: 2 * b + 2, :], axis=AxX, op=Alu.add,
                    apply_absolute_value=True,
                )
                Sb = sp.tile([128, 2], f32, tag="Sb")
                nc.vector.tensor_reduce(
                    Sb[:], wt4[:, 2 * b: 2 * b + 2, :], axis=AxX, op=Alu.add
                )
                Sh = sp.tile([128, 2], f32, tag="Sh")
                nc.gpsimd.tensor_scalar(Sh[:], Sb[:], 0.5, None, Alu.mult)

                G = sp.tile([3, A], f16, tag="G")
                for t in range(2):
                    Cc = sp.tile([128, 3], f16, tag="Cc")
                    nc.gpsimd.tensor_copy(
                        Cc[:, 0:1], csc[:, 2 * b + t: 2 * b + t + 1]
                    )
                    nc.vector.scalar_tensor_tensor(
                        Cc[:, 1:2], Tb[:, t: t + 1], 0.5, Sh[:, t: t + 1],
                        Alu.mult, Alu.add,
                    )
                    nc.vector.scalar_tensor_tensor(
                        Cc[:, 2:3], Tb[:, t: t + 1], 0.5, Sh[:, t: t + 1],
                        Alu.mult, Alu.subtract,
                    )
                    tpc = pmm.tile([3, 128], f16, tag="tp1")
                    nc.tensor.transpose(tpc[:], Cc[:], ident[:])
                    nc.vector.tensor_copy(
                        G[:, t * 128: (t + 1) * 128], tpc[:]
                    )

                ps_base = pbase.tile([U, A], f32, tag="psbase")
                nc.tensor.matmul(ps_base[:], F[:], G[:])

                wt = wt4[:, 2 * b: 2 * b + 2, :]
                adjT = sp.tile([128, 2, A], f32, tag="adjT")
                for at in range(2):
                    for jt in range(2):
                        ptr = pmm.tile([128, 128], f32, tag="tp1")
                        nc.tensor.transpose(
                            ptr[:], wt[:, at, jt * 128: (jt + 1) * 128],
                            identf[:],
                        )
                        nc.vector.tensor_scalar(
                            adjT[:, jt, at * 128: (at + 1) * 128],
                            ptr[:], 0.0, None, Alu.not_equal,
                        )
                embT = sp.tile([U, A], f32, tag="embT")
                nc.vector.tensor_scalar(
                    embT[:], ps_base[:], 0.0, None, Alu.max
                )
                EMBb = None
                for it in range(2):
                    nat = sp.tile([128, 2, U], f32, tag="nat")
                    for ht in range(2):
                        pnat = pmm.tile([128, U], f32, tag="tp1")
                        nc.tensor.transpose(
                            pnat[:], embT[:, ht * 128: (ht + 1) * 128],
                            identf[0:U, 0:U],
                        )
                        nc.vector.tensor_copy(nat[:, ht, :], pnat[:])
                    ps_y = pmm.tile([U, A], f32, tag="tp1")
                    nc.tensor.matmul(ps_y[:], nat[:, 0, :], adjT[:, 0, :],
                                     start=True, stop=False)
                    nc.tensor.matmul(ps_y[:], nat[:, 1, :], adjT[:, 1, :],
                                     start=False, stop=True)
                    ysb = sp.tile([U, A], f32, tag="ysb")
                    nc.vector.tensor_scalar(ysb[:], ps_y[:], INV_A, None,
                                            Alu.mult)
                    ps_it = pbase.tile([U, A], f32, tag="psbase")
                    nc.tensor.matmul(ps_it[:], F[:], G[:],
                                     start=True, stop=False)
                    nc.tensor.matmul(ps_it[:], wp[:, WP_W1:WP_W1 + 64],
                                     ysb[:], start=False, stop=True)
                    if it == 0:
                        embT = sp.tile([U, A], f32, tag="embT")
                        nc.vector.tensor_scalar(
                            embT[:], ps_it[:], 0.0, None, Alu.max
                        )
                    else:
                        EMBb = sp.tile([U, A], f16, tag="EMBb")
                        nc.vector.tensor_scalar(
                            EMBb[:], ps_it[:], 0.0, None, Alu.max
                        )

                sl = slice(b * A, (b + 1) * A)
                ph1 = phead.tile([HID, A], f32, tag="pmat")
                nc.tensor.matmul(ph1[:], aw1h, EMBb[:])
                h1 = sp.tile([HID, A], f16, tag="h1")
                nc.scalar.activation(h1[:], ph1[:], Act.Relu,
                                     bias=wp[:, WP_AB1:WP_AB1 + 1])
                ph2 = phead.tile([HID // 2, A], f32, tag="pmat")
                nc.tensor.matmul(ph2[:], aw2h, h1[:])
                h2 = sp.tile([HID // 2, A], f16, tag="h2")
                nc.vector.tensor_scalar(
                    h2[:], ph2[:], wp[0:32, WP_AB2:WP_AB2 + 1], 0.0,
                    Alu.add, op1=Alu.max,
                )
                pa = phead.tile([1, A], f32, tag="pa")
                nc.tensor.matmul(pa[:], aw3h, h2[:])

                phv = phead.tile([HID, A], f32, tag="pmat")
                nc.tensor.matmul(phv[:], vw1h, EMBb[:])
                hv = sp.tile([HID, A], f32, tag="hv")
                hv_cs = sp.tile([U, 1], f32, tag="hv_cs")
                nc.scalar.activation(hv[:], phv[:], Act.Relu,
                                     bias=wp[:, WP_VB1:WP_VB1 + 1],
                                     accum_out=hv_cs[:])
                psv = phead.tile([1, 1], f32, tag="pa")
                nc.tensor.matmul(psv[:], hv_cs[:], wp[:, WP_VW2:WP_VW2 + 1])

                ra = sp.tile([1, 1], f32, tag="ra")
                nc.vector.tensor_reduce(ra[:], pa[:], axis=AxX, op=Alu.add)
                Kb = sp.tile([1, 1], f32, tag="Kb")
                nc.vector.tensor_scalar(
                    Kb[:], psv[:], ra[:], kc[:], Alu.subtract, op1=Alu.add
                )
                nc.vector.scalar_tensor_tensor(
                    FIN[:, sl], pa[:], Kb[:], m10[:, sl], Alu.add, Alu.add
                )
                if b == 0:
                    nc.sync.dma_start(out[b, :][None, :], FIN[:, sl])
                else:
                    nc.scalar.dma_start(out[b, :][None, :], FIN[:, sl])

    return nc


_NC_CACHE: dict[bool, bass.Bass] = {}


def _get_nc(fast: bool) -> bass.Bass:
    if fast not in _NC_CACHE:
        nc = _build_fast() if fast else _build_general()
        nc.finalize()
        _NC_CACHE[fast] = nc
    return _NC_CACHE[fast]


def _make_in_maps_fast(inputs: dict) -> list[dict]:
    x = np.asarray(inputs["x"], dtype=np.float32)
    w16 = np.ascontiguousarray(x[:, 2:A + 2, :].astype(np.float16))
    cur16 = np.ascontiguousarray(x[:, 0, :].astype(np.float16))
    msk16 = np.ascontiguousarray(x[:, 1, :].astype(np.float16))
    wpk = _pack_fast_weights(
        {k: np.asarray(inputs[k], dtype=np.float32) for k in WEIGHT_NAMES}
    )
    in_maps = []
    for c in range(NCORES):
        s = slice(c * BPC, (c + 1) * BPC)
        in_maps.append({
            "w16": np.ascontiguousarray(w16[s]),
            "aux_c": cur16[s].reshape(1, BPC * A),
            "aux_m": msk16[s].reshape(1, BPC * A),
            "wpk": wpk,
        })
    return in_maps


def _make_in_maps_general(inputs: dict) -> list[dict]:
    x = np.ascontiguousarray(np.asarray(inputs["x"], dtype=np.float32))
    wpk = _pack_weights(
        {k: np.asarray(inputs[k], dtype=np.float32) for k in WEIGHT_NAMES}
    )
    in_maps = []
    for c in range(NCORES):
        in_maps.append({
            "xs": np.ascontiguousarray(x[c * BPC: (c + 1) * BPC]),
            "wpack": wpk,
        })
    return in_maps


def run(inputs: dict, trace: bool = False, tmpdir: str | None = None):
    """Returns (output [B, A] f32, BassKernelResults)."""
    x = np.asarray(inputs["x"])
    fast = bool((x[:, 2:, :] != 0.0).all())
    nc = _get_nc(fast)
    in_maps = _make_in_maps_fast(inputs) if fast \
        else _make_in_maps_general(inputs)
    res = run_bass_kernel_spmd(
        nc, in_maps, list(range(NCORES)), trace=trace, tmpdir=tmpdir,
    )
    out = np.concatenate(
        [res.results[i]["out"].reshape(BPC, A) for i in range(NCORES)], axis=0
    )
    return out, res


def kernel(**inputs) -> np.ndarray:
    out, _ = run(inputs)
    return out


# revision 22
# speedup vs baseline: 1.1439x; 1.1439x over previous
"""Trainium2 Bass kernel for nn_DQN_9904194584789 (GNN message passing DQN).

Reference math (B=16, A=256, U=64, T=3):
    cur_sol = x[:,0,:]; mask = x[:,1,:]; w = x[:,2:,:]          # [B,A,A]
    adj = (w != 0)
    e1 = cur_sol[:,:,None] @ W0                                  # rank-1
    e3 = (sum_j relu(w[...,None] * W3) / A) @ W2                 # collapses:
         sum_j relu(w*c) = P*relu(c) + N*relu(-c) elementwise in c, with
         P = sum_j relu(w), N = sum_j relu(-w) = -M, M = sum_j min(w, 0)
         => e3 = P x (relu(W3)@W2/A) + M x (-relu(-W3)@W2/A) = P x Fp + M x Fn
    base = e1 + e3 (loop invariant);  emb_1 = relu(base)
    emb_{t+1} = relu(base + c_t) with c_t = W1^T @ colsum(emb_t)/A   (fast
    path: w has no exact zeros so adj is all-ones and adj@emb collapses to
    a colsum broadcast).
    heads: dueling MLP on emb_3; out = pa + [sum(psv) - sum(pa) + kc] + 10*mask
    with kc = A*v_b2 - (A-1)*a_b3.

Sharding: pure data-parallel over batch B (2 batches per core x 8 cores),
replicated weights host-packed into one [128, NW] buffer (fp16 payloads as
raw bits in f32 columns, incl. a host-packed fp16 identity).  Host work is
only slicing / layout packing / dtype casts (x cast to fp16); all model math
runs on device.

Fast-path structure (both batches fused into [., 512] ops):
  - P/M row-sum stats via tensor_scalar relu/min chunk ops with accum_out
    (DVE 5 chunks + Pool 3 chunks in parallel), cast once to fp16, PE
    transposes -> G [2, 512]
  - base = Fts^T @ G + W0row^T @ cur_row (two matmuls into one PSUM bank)
  - 3 relu iterations on DVE reading base straight from PSUM, with colsum
    accum_out; c = W1^T cs/A via fp16 hi+lo matmuls
  - dueling heads: row-sum corrections come free from accum_out on the
    h2/hv relu legs + two tiny matmuls (no big reduces); 10*mask is a
    rank-1 fp16 matmul accumulated into the pa PSUM bank; kc folds into K.

General path (any exact zero in w): the original f32 kernel with real
adjacency matmuls, unchanged.
"""

import numpy as np
from contextlib import ExitStack

import concourse.bass as bass
import concourse.bacc as bacc
import concourse.tile as tile
from concourse import mybir
from concourse.bass_utils import run_bass_kernel_spmd
from concourse.masks import make_identity

f32 = mybir.dt.float32
f16 = mybir.dt.float16
Alu = mybir.AluOpType
Act = mybir.ActivationFunctionType
AxX = mybir.AxisListType.X

B, A, U, HID = 16, 256, 64, 64
NCORES = 8
BPC = B // NCORES  # batches per core
INV_A = 1.0 / A

WEIGHT_NAMES = [
    "W0", "W1", "W2", "W3",
    "a_w1", "a_b1", "a_w2", "a_b2", "a_w3", "a_b3",
    "v_w1", "v_b1", "v_w2", "v_b2",
]

# ---------------------------------------------------------------------------
# fast-path wpk layout: [128, NW] f32; fp16 payloads packed pairwise as bits.
_c = 0
def _adv(n):
    global _c
    r = _c
    _c += n
    return r

CW1H = _adv(32)     # (W1/A) fp16 hi      [64, 64]
CW1L = _adv(32)     # (W1/A) fp16 lo      [64, 64]
CW2H = _adv(32)     # W2 fp16             [64, 64]
CW3H = _adv(1)      # W3 col fp16         [64, 1]
CAW1 = _adv(32)     # a_w1 fp16           [64, 64]
CAW2 = _adv(16)     # a_w2 fp16           [64, 32]
CVW1 = _adv(32)     # v_w1 fp16           [64, 64]
CW0R = _adv(32)     # W0 row fp16         [1, 64] (partition 0)
CA3M = _adv(1)      # [a_w3; 10.0] fp16   [33, 1]
CVW2H = _adv(1)     # v_w2 fp16 col       [64, 1]
CA3NH = _adv(1)     # -a_w3 fp16 col      [32, 1]
CAB1 = _adv(1)      # a_b1 f32 col        [64, 1]
CVB1 = _adv(1)      # v_b1 f32 col        [64, 1]
CAB2 = _adv(1)      # a_b2 f32 col        [32, 1]
CAB3 = _adv(1)      # a_b3 scalar         [1, 1]
CVB2 = _adv(1)      # v_b2 scalar         [1, 1]
CIDT = _adv(64)     # identity fp16       [128, 128]
NW = _c


def _pack_fast_weights(inputs: dict) -> np.ndarray:
    wp = np.zeros((128, NW), dtype=np.float32)

    def place(col, arr):  # fp16 bits packed pairwise into f32 columns
        raw = np.ascontiguousarray(
            np.asarray(arr, np.float32).astype(np.float16)
        ).view(np.uint16)
        k = raw.shape[1]
        pad = np.zeros((raw.shape[0], (k + 1) // 2 * 2), np.uint16)
        pad[:, :k] = raw
        fview = pad.view(np.float32)
        wp[:fview.shape[0], col:col + fview.shape[1]] = fview

    # 1/A = 1/256 is a power of two: W1/A is an exact f32 re-encoding;
    # hi+lo fp16 split keeps the W1 product near-f32 accurate.
    w1a = np.asarray(inputs["W1"], np.float32) * INV_A
    w1ah = w1a.astype(np.float16).astype(np.float32)
    place(CW1H, w1a)
    place(CW1L, w1a - w1ah)
    place(CW2H, inputs["W2"])
    place(CW3H, np.asarray(inputs["W3"], np.float32).T)   # [64, 1]
    place(CAW1, inputs["a_w1"])
    place(CAW2, inputs["a_w2"])
    place(CVW1, inputs["v_w1"])
    place(CW0R, inputs["W0"])                              # [1, 64] row
    a3m = np.zeros((33, 1), np.float32)
    a3m[:32, 0] = np.asarray(inputs["a_w3"], np.float32)[:, 0]
    a3m[32, 0] = 10.0
    place(CA3M, a3m)
    place(CVW2H, np.asarray(inputs["v_w2"], np.float32))
    place(CA3NH, -np.asarray(inputs["a_w3"], np.float32))
    place(CIDT, np.eye(128, dtype=np.float32))
    wp[:64, CAB1] = inputs["a_b1"]
    wp[:64, CVB1] = inputs["v_b1"]
    wp[:32, CAB2] = inputs["a_b2"]
    wp[0, CAB3] = inputs["a_b3"][0]
    wp[0, CVB2] = inputs["v_b2"][0]
    return wp


def _build_fast() -> bass.Bass:
    # Bacc (not raw Bass): its finalize() runs move_matmul_waits_to_ldweights
    # + generate_event_semaphores for the TRN2 one-wait-per-inst constraint.
    nc = bacc.Bacc(
        "TRN2", target_bir_lowering=False, debug=False, num_devices=NCORES
    )
    w16d = nc.declare_dram_parameter("w16", [BPC, A, A], f16, isOutput=False)
    auxc = nc.declare_dram_parameter("aux_c", [1, BPC * A], f16, isOutput=False)
    auxm = nc.declare_dram_parameter("aux_m", [1, BPC * A], f16, isOutput=False)
    wpd = nc.declare_dram_parameter("wpk", [128, NW], f32, isOutput=False)
    outd = nc.declare_dram_parameter("out", [1, BPC * A], f32, isOutput=True)

    NBA = BPC * A  # 512

    with tile.TileContext(nc) as tc, ExitStack() as ctx, \
         nc.allow_low_precision("fp16 colsum accums; tol 5e-3"):
        cp = ctx.enter_context(tc.tile_pool(name="const", bufs=1))
        sp = ctx.enter_context(tc.tile_pool(name="scratch", bufs=2))
        dv = ctx.enter_context(tc.tile_pool(name="dumpv", bufs=2))
        dp = ctx.enter_context(tc.tile_pool(name="dumpp", bufs=2))

        # ---------- input DMAs, issued first on distinct queues -----------
        # batch1 feeds the (slower) ACT chain -> land it first on sync.
        wt4 = cp.tile([128, 2 * BPC, A], f16, tag="wt4")
        nc.sync.dma_start(
            wt4[:, 2:4, :],
            w16d[1, :, :].rearrange("(t p) j -> p t j", p=128),
        )
        wp = cp.tile([128, NW], f32, tag="wp")
        nc.scalar.dma_start(wp[:], wpd[:])
        nc.scalar.dma_start(
            wt4[:, 0:2, :],
            w16d[0, :, :].rearrange("(t p) j -> p t j", p=128),
        )
        cur = cp.tile([1, NBA], f16, tag="cur")
        nc.gpsimd.dma_start(cur[:], auxc[:])
        # per-batch h2 tiles [33, 256]: partition 32 carries the mask row so
        # the pa matmul computes out_a + 10*mask in one [33,1]^T @ [33,256].
        h2A = cp.tile([HID // 2 + 1, A], f16, tag="h2A")
        nc.gpsimd.dma_start(h2A[32:33, :], auxm[:, 0:A])
        h2B = cp.tile([HID // 2 + 1, A], f16, tag="h2B")
        nc.gpsimd.dma_start(h2B[32:33, :], auxm[:, A:2 * A])

        # views of host-packed weights
        w1hh = wp[0:64, CW1H:CW1H + 32].bitcast(f16)          # [64, 64]
        w1lh = wp[0:64, CW1L:CW1L + 32].bitcast(f16)          # [64, 64]
        w2h = wp[0:64, CW2H:CW2H + 32].bitcast(f16)           # [64, 64]
        w3h = wp[0:64, CW3H:CW3H + 1].bitcast(f16)[:, 0:1]    # [64, 1]
        aw1h = wp[0:64, CAW1:CAW1 + 32].bitcast(f16)          # [64, 64]
        aw2h = wp[0:64, CAW2:CAW2 + 16].bitcast(f16)          # [64, 32]
        vw1h = wp[0:64, CVW1:CVW1 + 32].bitcast(f16)          # [64, 64]
        w0r = wp[0:1, CW0R:CW0R + 32].bitcast(f16)            # [1, 64]
        a3m = wp[0:33, CA3M:CA3M + 1].bitcast(f16)[:, 0:1]    # [33, 1]
        vw2h = wp[0:64, CVW2H:CVW2H + 1].bitcast(f16)[:, 0:1]
        a3nh = wp[0:32, CA3NH:CA3NH + 1].bitcast(f16)[:, 0:1]
        ident = wp[:, CIDT:CIDT + 64].bitcast(f16)            # [128, 128]
        ab1f = wp[0:64, CAB1:CAB1 + 1]
        vb1f = wp[0:64, CVB1:CVB1 + 1]
        ab2f = wp[0:32, CAB2:CAB2 + 1]

        with tc.tile_pool(name="pmm", bufs=2, space="PSUM") as pmm, \
             tc.tile_pool(name="pbase", bufs=1, space="PSUM") as pbase, \
             tc.tile_pool(name="phead", bufs=1, space="PSUM") as phead:

            # ---------- setup (runs in the input-DMA shadow) --------------
            # e3 = T x (|W3|@W2/2A) + S x (W3@W2/2A) with T = sum|w|,
            # S = sum w  (from e3 = P*Fp + M*Fn, P=(S+T)/2, M=(S-T)/2).
            w3p = cp.tile([U, 1], f16, tag="w3p")
            nc.scalar.activation(w3p[:], w3h, Act.Abs, scale=0.5 * INV_A)
            w3i = cp.tile([U, 1], f16, tag="w3i")
            nc.scalar.activation(w3i[:], w3h, Act.Identity, scale=0.5 * INV_A)
            psF2 = pmm.tile([U, 2], f32, tag="tp")
            nc.tensor.matmul(psF2[:, 0:1], w2h, w3p[:])
            nc.tensor.matmul(psF2[:, 1:2], w2h, w3i[:])
            Fc16 = cp.tile([U, 2], f16, tag="Fc16")
            nc.scalar.activation(Fc16[:], psF2[:], Act.Identity)
            psFT = pmm.tile([2, U], f16, tag="tp")
            nc.tensor.transpose(psFT[:], Fc16[:], ident[0:U, 0:U])
            Fts = cp.tile([2, U], f16, tag="Fts")
            nc.scalar.activation(Fts[:], psFT[:], Act.Identity)

            # kc = A*v_b2 - (A-1)*a_b3 (scalar part of the dueling combine)
            t256 = cp.tile([1, 1], f32, tag="t256")
            nc.gpsimd.tensor_scalar(
                t256[:], wp[0:1, CVB2:CVB2 + 1], float(A), None, Alu.mult
            )
            kc = cp.tile([1, 1], f32, tag="kc")
            nc.gpsimd.tensor_scalar(
                kc[:], wp[0:1, CAB3:CAB3 + 1], -float(A - 1), t256[:],
                Alu.mult, Alu.add,
            )
            zz = cp.tile([U, A], f16, tag="zz")
            nc.gpsimd.memset(zz[:], 0.0)

            # ---------- T/S stats -> G -> base, per batch (B first) -------
            # batch1's data lands first and feeds the slower ACT chain.
            TScB = cp.tile([128, 4], f16, tag="TScB")
            with nc.allow_low_precision("fp16 row-sum stats; tol 5e-3"):
                nc.vector.tensor_reduce(
                    TScB[:, 0:4:2], wt4[:, 2:4, :], axis=AxX,
                    op=Alu.add, apply_absolute_value=True,
                )
                nc.vector.tensor_reduce(
                    TScB[:, 1:4:2], wt4[:, 2:4, :], axis=AxX,
                    op=Alu.add
                )
            GB = cp.tile([2, 2, 128], f16, tag="GB")
            psT = pmm.tile([2, 2, 128], f16, tag="tp")
            nc.tensor.transpose(psT[:, 0, :], TScB[:, 0:2], ident[:])
            nc.tensor.transpose(psT[:, 1, :], TScB[:, 2:4], ident[:])
            nc.vector.tensor_copy(GB[:], psT[:])
            ps_bB = pbase.tile([U, A], f32, tag="psbB")
            nc.tensor.matmul(ps_bB[:], Fts[:],
                             GB[:].rearrange("r c p -> r (c p)"),
                             start=True, stop=False)
            nc.tensor.matmul(ps_bB[:], w0r, cur[:, A:2 * A], start=False,
                             stop=True)

            TScA = cp.tile([128, 4], f16, tag="TScA")
            with nc.allow_low_precision("fp16 row-sum stats; tol 5e-3"):
                nc.vector.tensor_reduce(
                    TScA[:, 0:4:2], wt4[:, 0:2, :], axis=AxX,
                    op=Alu.add, apply_absolute_value=True,
                )
                nc.vector.tensor_reduce(
                    TScA[:, 1:4:2], wt4[:, 0:2, :], axis=AxX,
                    op=Alu.add
                )
            GA = cp.tile([2, 2, 128], f16, tag="GA")
            psT = pmm.tile([2, 2, 128], f16, tag="tp")
            nc.tensor.transpose(psT[:, 0, :], TScA[:, 0:2], ident[:])
            nc.tensor.transpose(psT[:, 1, :], TScA[:, 2:4], ident[:])
            nc.vector.tensor_copy(GA[:], psT[:])
            ps_bA = pbase.tile([U, A], f32, tag="psbA")
            nc.tensor.matmul(ps_bA[:], Fts[:],
                             GA[:].rearrange("r c p -> r (c p)"),
                             start=True, stop=False)
            nc.tensor.matmul(ps_bA[:], w0r, cur[:, 0:A], start=False,
                             stop=True)

            # ---------- message passing ----------------------------------
            # independent per-batch chains: batch0 on DVE, batch1 on ACT;
            # each chain is engine-self-contained (PSUM banks and scratch
            # tiles are single-engine to avoid serializing hazards).
            cs_a = sp.tile([U, 1], f16, tag="cs_a")
            cs_b = sp.tile([U, 1], f16, tag="cs_b")
            dm = dp.tile([U, A], f16, tag="dms")
            nc.scalar.activation(dm[:], ps_bB[:], Act.Relu,
                                 accum_out=cs_b[:])
            dm = dv.tile([U, A], f16, tag="dme")
            nc.vector.tensor_scalar(
                dm[:], ps_bA[:], 0.0, None, Alu.max,
                op1=Alu.add, accum_out=cs_a[:],
            )
            emb_a = cp.tile([U, A], f16, tag="emb_a")
            emb_b = cp.tile([U, A], f16, tag="emb_b")
            for it in range(2):
                psCB = pmm.tile([U, 1], f32, tag="tp")
                nc.tensor.matmul(psCB[:], w1hh, cs_b[:])
                csbB = sp.tile([U, 1], f32, tag="csbB")
                nc.scalar.activation(csbB[:], psCB[:], Act.Identity)
                psCA = pmm.tile([U, 1], f32, tag="tp")
                nc.tensor.matmul(psCA[:], w1hh, cs_a[:])
                csbA = sp.tile([U, 1], f32, tag="csbA")
                nc.vector.tensor_copy(csbA[:], psCA[:])
                if it == 0:
                    cs_a = sp.tile([U, 1], f16, tag="cs_a2")
                    cs_b = sp.tile([U, 1], f16, tag="cs_b2")
                    dm = dp.tile([U, A], f16, tag="dms")
                    nc.scalar.activation(dm[:], ps_bB[:], Act.Relu,
                                         bias=csbB[:], accum_out=cs_b[:])
                    dm = dv.tile([U, A], f16, tag="dme")
                    nc.vector.scalar_tensor_tensor(
                        dm[:], ps_bA[:], csbA[:], zz[:],
                        Alu.add, Alu.max, accum_out=cs_a[:],
                    )
                else:
                    nc.scalar.activation(emb_b[:], ps_bB[:],
                                         Act.Relu, bias=csbB[:])
                    nc.vector.tensor_scalar(
                        emb_a[:], ps_bA[:], csbA[:], 0.0,
                        Alu.add, op1=Alu.max,
                    )

            # ---------- dueling heads: per-batch chains (B first) ---------
            ph1B = phead.tile([HID, A], f32, tag="hb")
            nc.tensor.matmul(ph1B[:], aw1h, emb_b[:])
            phvB = phead.tile([HID, A], f32, tag="vb")
            nc.tensor.matmul(phvB[:], vw1h, emb_b[:])
            h1B = cp.tile([HID, A], f16, tag="h1B")
            nc.scalar.activation(h1B[:], ph1B[:], Act.Relu, bias=ab1f)

            ph1A = phead.tile([HID, A], f32, tag="ha")
            nc.tensor.matmul(ph1A[:], aw1h, emb_a[:])
            phvA = phead.tile([HID, A], f32, tag="va")
            nc.tensor.matmul(phvA[:], vw1h, emb_a[:])
            h1A = cp.tile([HID, A], f16, tag="h1A")
            nc.vector.tensor_scalar(
                h1A[:], ph1A[:], ab1f, 0.0, Alu.add, op1=Alu.max,
            )

            ph2B = phead.tile([HID // 2, A], f32, tag="hb")
            nc.tensor.matmul(ph2B[:], aw2h, h1B[:])
            hvcsb = sp.tile([U, 1], f16, tag="hvcsb")
            dmv = dp.tile([U, A], f16, tag="dms")
            nc.scalar.activation(dmv[:], phvB[:], Act.Relu, bias=vb1f,
                                 accum_out=hvcsb[:])
            h2csb = sp.tile([HID // 2, 1], f16, tag="h2csb")
            nc.scalar.activation(h2B[0:32, :], ph2B[:], Act.Relu,
                                 bias=ab2f, accum_out=h2csb[:])

            ph2A = phead.tile([HID // 2, A], f32, tag="ha")
            nc.tensor.matmul(ph2A[:], aw2h, h1A[:])
            hvcsa = sp.tile([U, 1], f16, tag="hvcsa")
            dmv = dv.tile([U, A], f16, tag="dme")
            nc.vector.scalar_tensor_tensor(
                dmv[:], phvA[:], vb1f, zz[:], Alu.add, Alu.max,
                accum_out=hvcsa[:],
            )
            h2csa = sp.tile([HID // 2, 1], f16, tag="h2csa")
            nc.vector.scalar_tensor_tensor(
                h2A[0:32, :], ph2A[:], ab2f, zz[0:32, :],
                Alu.add, Alu.max, accum_out=h2csa[:],
            )

            # K columns: fp16 casts of the accums feed tiny PE matmuls
            psK1 = pmm.tile([1, 1], f32, tag="tp")
            nc.tensor.matmul(psK1[:], vw2h, hvcsb[:], start=True, stop=False)
            nc.tensor.matmul(psK1[:], a3nh, h2csb[:], start=False, stop=True)
            paB = phead.tile([1, A], f32, tag="hb")
            nc.tensor.matmul(paB[:], a3m, h2B[:])
            K1 = sp.tile([1, 1], f32, tag="K1")
            nc.vector.tensor_scalar(K1[:], psK1[:], kc[0:1, 0:1], None,
                                    Alu.add)
            FINb = cp.tile([1, A], f32, tag="FINb")
            nc.scalar.activation(FINb[:], paB[:], Act.Identity,
                                 bias=K1[0:1, 0:1])
            nc.scalar.dma_start(outd[:, A:2 * A], FINb[:])

            psK0 = pmm.tile([1, 1], f32, tag="tp")
            nc.tensor.matmul(psK0[:], vw2h, hvcsa[:], start=True, stop=False)
            nc.tensor.matmul(psK0[:], a3nh, h2csa[:], start=False, stop=True)
            paA = phead.tile([1, A], f32, tag="ha")
            nc.tensor.matmul(paA[:], a3m, h2A[:])
            K0 = sp.tile([1, 1], f32, tag="K0")
            nc.vector.tensor_scalar(K0[:], psK0[:], kc[0:1, 0:1], None,
                                    Alu.add)
            FINa = cp.tile([1, A], f32, tag="FINa")
            nc.vector.tensor_scalar(
                FINa[:], paA[:], K0[0:1, 0:1], None, Alu.add
            )
            nc.sync.dma_start(outd[:, 0:A], FINa[:])

    return nc


# ---------------------------------------------------------------------------
# General path (exact zeros in w): original f32 kernel, unchanged.

WP_W1 = 0
WP_W2 = 64
WP_W3 = 128
WP_AB1 = 129
WP_VB1 = 130
WP_AB2 = 131
WP_VW2 = 132
WP_AB3 = 133
WP_VB2 = 134
WP_AW1H = 135
WP_AW2H = 167
WP_AW3H = 183
WP_VW1H = 184
WP_W0C = 216
WP_W1HH = 217
WP_W1LH = 249
WP_AW3F = 280
WP_W2H = 281
NWP = 313


def _pack_weights(inputs: dict) -> np.ndarray:
    wp = np.zeros((64, NWP), dtype=np.float32)
    wp[:, WP_W1:WP_W1 + 64] = inputs["W1"]
    wp[:, WP_W2:WP_W2 + 64] = inputs["W2"]
    wp[:, WP_W3] = inputs["W3"][0]
    wp[:, WP_AB1] = inputs["a_b1"]
    wp[:, WP_VB1] = inputs["v_b1"]
    wp[:32, WP_AB2] = inputs["a_b2"]
    wp[:, WP_VW2] = inputs["v_w2"][:, 0]
    wp[0, WP_AB3] = inputs["a_b3"][0]
    wp[0, WP_VB2] = inputs["v_b2"][0]
    wp[:32, WP_AW3F] = inputs["a_w3"][:, 0]

    def place(col, arr):
        raw = np.ascontiguousarray(
            np.asarray(arr, np.float32).astype(np.float16)
        ).view(np.uint16)
        k = raw.shape[1]
        pad = np.zeros((raw.shape[0], (k + 1) // 2 * 2), np.uint16)
        pad[:, :k] = raw
        fview = pad.view(np.float32)
        wp[:fview.shape[0], col:col + fview.shape[1]] = fview

    place(WP_AW1H, inputs["a_w1"])
    place(WP_AW2H, inputs["a_w2"])
    place(WP_AW3H, inputs["a_w3"][:, 0:1])
    place(WP_VW1H, inputs["v_w1"])
    place(WP_W0C, inputs["W0"].T)
    place(WP_W2H, inputs["W2"])
    w1 = np.asarray(inputs["W1"], np.float32)
    w1h = w1.astype(np.float16).astype(np.float32)
    place(WP_W1HH, w1)
    place(WP_W1LH, w1 - w1h)
    return wp


def _build_general() -> bass.Bass:
    nc = bacc.Bacc(
        "TRN2", target_bir_lowering=False, debug=False, num_devices=NCORES
    )
    xs = nc.declare_dram_parameter("xs", [BPC, A + 2, A], f32, isOutput=False)
    wpd = nc.declare_dram_parameter("wpack", [64, NWP], f32, isOutput=False)
    out = nc.declare_dram_parameter("out", [BPC, A], f32, isOutput=True)

    with tile.TileContext(nc) as tc, ExitStack() as ctx, \
         nc.allow_low_precision("fp16 colsum accums; tol 5e-3"):
        cp = ctx.enter_context(tc.tile_pool(name="const", bufs=1))
        sp = ctx.enter_context(tc.tile_pool(name="scratch", bufs=2))

        wp = cp.tile([64, NWP], f32, tag="wp")
        nc.sync.dma_start(wp[:], wpd[:])
        wt4 = cp.tile([128, 2 * BPC, A], f32, tag="wt4")
        for b in range(BPC):
            nc.scalar.dma_start(
                wt4[:, 2 * b: 2 * b + 2, :],
                xs[b, 2: A + 2, :].rearrange("(t p) j -> p t j", p=128),
            )
        csc = cp.tile([128, 2 * BPC], f32, tag="csc")
        for b in range(BPC):
            nc.gpsimd.dma_start(
                csc[:, 2 * b: 2 * b + 2],
                xs[b, 0, :].rearrange("(t p) -> p t", p=128),
            )
        mrow = cp.tile([1, BPC * A], f32, tag="mrow")
        nc.gpsimd.dma_start(
            mrow[:].rearrange("p (b a) -> p b a", b=BPC),
            xs[:, 1, :][None, :, :],
        )

        aw1h = wp[:, WP_AW1H:WP_AW1H + 32].bitcast(f16)
        aw2h = wp[:, WP_AW2H:WP_AW2H + 16].bitcast(f16)
        aw3h = wp[0:32, WP_AW3H:WP_AW3H + 1].bitcast(f16)[:, 0:1]
        vw1h = wp[:, WP_VW1H:WP_VW1H + 32].bitcast(f16)
        w0c = wp[:, WP_W0C:WP_W0C + 1].bitcast(f16)[:, 0:1]

        ident = cp.tile([128, 128], f16, tag="ident")
        make_identity(nc, ident[:])
        identf = cp.tile([128, 128], f32, tag="identf")
        make_identity(nc, identf[:])

        with tc.tile_pool(name="psetup", bufs=2, space="PSUM") as psetup:
            w2h = wp[:, WP_W2H:WP_W2H + 32].bitcast(f16)
            w3p = cp.tile([U, 1], f16, tag="w3p")
            nc.scalar.activation(w3p[:], wp[:, WP_W3:WP_W3 + 1], Act.Relu)
            w3n = cp.tile([U, 1], f16, tag="w3n")
            nc.scalar.activation(w3n[:], wp[:, WP_W3:WP_W3 + 1], Act.Relu,
                                 scale=-1.0)
            Fc = cp.tile([U, 3], f16, tag="Fc")
            nc.vector.tensor_copy(Fc[:, 0:1], w0c)
            pspc = psetup.tile([U, 1], f32, tag="pscol")
            nc.tensor.matmul(pspc[:], w2h, w3p[:])
            nc.scalar.mul(Fc[:, 1:2], pspc[:], INV_A)
            psnc = psetup.tile([U, 1], f32, tag="pscol")
            nc.tensor.matmul(psnc[:], w2h, w3n[:])
            nc.scalar.mul(Fc[:, 2:3], psnc[:], INV_A)
            psF = psetup.tile([3, U], f16, tag="psF")
            nc.tensor.transpose(psF[:], Fc[:], ident[0:U, 0:U])
            F = cp.tile([3, U], f16, tag="F")
            nc.vector.tensor_copy(F[:], psF[:])

        t256 = cp.tile([1, 1], f32, tag="t256")
        nc.gpsimd.tensor_scalar(
            t256[:], wp[0:1, WP_VB2:WP_VB2 + 1], float(A), None, Alu.mult
        )
        kc = cp.tile([1, 1], f32, tag="kc")
        nc.gpsimd.tensor_scalar(
            kc[:], wp[0:1, WP_AB3:WP_AB3 + 1], -float(A - 1), t256[:],
            Alu.mult, Alu.add,
        )

        m10 = cp.tile([1, BPC * A], f32, tag="m10")
        nc.scalar.mul(m10[:], mrow[:], 10.0)

        FIN = cp.tile([1, BPC * A], f32, tag="FIN")

        with tc.tile_pool(name="pmm", bufs=1, space="PSUM") as pmm, \
             tc.tile_pool(name="pbase", bufs=2, space="PSUM") as pbase, \
             tc.tile_pool(name="phead", bufs=2, space="PSUM") as phead:
            for b in range(BPC):
                Tb = sp.tile([128, 2], f32, tag="Tb")
                nc.vector.tensor_reduce(
                    Tb[:], wt4[:, 2 * b: 2 * b + 2, :], axis=AxX, op=Alu.add,
                    apply_absolute_value=True,
                )
                Sb = sp.tile([128, 2], f32, tag="Sb")
                nc.vector.tensor_reduce(
                    Sb[:], wt4[:, 2 * b: 2 * b + 2, :], axis=AxX, op=Alu.add
                )
                Sh = sp.tile([128, 2], f32, tag="Sh")
                nc.gpsimd.tensor_scalar(Sh[:], Sb[:], 0.5, None, Alu.mult)

                G = sp.tile([3, A], f16, tag="G")
                for t in range(2):
                    Cc = sp.tile([128, 3], f16, tag="Cc")
                    nc.gpsimd.tensor_copy(
                        Cc[:, 0:1], csc[:, 2 * b + t: 2 * b + t + 1]
                    )
                    nc.vector.scalar_tensor_tensor(
                        Cc[:, 1:2], Tb[:, t: t + 1], 0.5, Sh[:, t: t + 1],
                        Alu.mult, Alu.add,
                    )
                    nc.vector.scalar_tensor_tensor(
                        Cc[:, 2:3], Tb[:, t: t + 1], 0.5, Sh[:, t: t + 1],
                        Alu.mult, Alu.subtract,
                    )
                    tpc = pmm.tile([3, 128], f16, tag="tp1")
                    nc.tensor.transpose(tpc[:], Cc[:], ident[:])
                    nc.vector.tensor_copy(
                        G[:, t * 128: (t + 1) * 128], tpc[:]
                    )

                ps_base = pbase.tile([U, A], f32, tag="psbase")
                nc.tensor.matmul(ps_base[:], F[:], G[:])

                wt = wt4[:, 2 * b: 2 * b + 2, :]
                adjT = sp.tile([128, 2, A], f32, tag="adjT")
                for at in range(2):
                    for jt in range(2):
                        ptr = pmm.tile([128, 128], f32, tag="tp1")
                        nc.tensor.transpose(
                            ptr[:], wt[:, at, jt * 128: (jt + 1) * 128],
                            identf[:],
                        )
                        nc.vector.tensor_scalar(
                            adjT[:, jt, at * 128: (at + 1) * 128],
                            ptr[:], 0.0, None, Alu.not_equal,
                        )
                embT = sp.tile([U, A], f32, tag="embT")
                nc.vector.tensor_scalar(
                    embT[:], ps_base[:], 0.0, None, Alu.max
                )
                EMBb = None
                for it in range(2):
                    nat = sp.tile([128, 2, U], f32, tag="nat")
                    for ht in range(2):
                        pnat = pmm.tile([128, U], f32, tag="tp1")
                        nc.tensor.transpose(
                            pnat[:], embT[:, ht * 128: (ht + 1) * 128],
                            identf[0:U, 0:U],
                        )
                        nc.vector.tensor_copy(nat[:, ht, :], pnat[:])
                    ps_y = pmm.tile([U, A], f32, tag="tp1")
                    nc.tensor.matmul(ps_y[:], nat[:, 0, :], adjT[:, 0, :],
                                     start=True, stop=False)
                    nc.tensor.matmul(ps_y[:], nat[:, 1, :], adjT[:, 1, :],
                                     start=False, stop=True)
                    ysb = sp.tile([U, A], f32, tag="ysb")
                    nc.vector.tensor_scalar(ysb[:], ps_y[:], INV_A, None,
                                            Alu.mult)
                    ps_it = pbase.tile([U, A], f32, tag="psbase")
                    nc.tensor.matmul(ps_it[:], F[:], G[:],
                                     start=True, stop=False)
                    nc.tensor.matmul(ps_it[:], wp[:, WP_W1:WP_W1 + 64],
                                     ysb[:], start=False, stop=True)
                    if it == 0:
                        embT = sp.tile([U, A], f32, tag="embT")
                        nc.vector.tensor_scalar(
                            embT[:], ps_it[:], 0.0, None, Alu.max
                        )
                    else:
                        EMBb = sp.tile([U, A], f16, tag="EMBb")
                        nc.vector.tensor_scalar(
                            EMBb[:], ps_it[:], 0.0, None, Alu.max
                        )

                sl = slice(b * A, (b + 1) * A)
                ph1 = phead.tile([HID, A], f32, tag="pmat")
                nc.tensor.matmul(ph1[:], aw1h, EMBb[:])
                h1 = sp.tile([HID, A], f16, tag="h1")
                nc.scalar.activation(h1[:], ph1[:], Act.Relu,
                                     bias=wp[:, WP_AB1:WP_AB1 + 1])
                ph2 = phead.tile([HID // 2, A], f32, tag="pmat")
                nc.tensor.matmul(ph2[:], aw2h, h1[:])
                h2 = sp.tile([HID // 2, A], f16, tag="h2")
                nc.vector.tensor_scalar(
                    h2[:], ph2[:], wp[0:32, WP_AB2:WP_AB2 + 1], 0.0,
                    Alu.add, op1=Alu.max,
                )
                pa = phead.tile([1, A], f32, tag="pa")
                nc.tensor.matmul(pa[:], aw3h, h2[:])

                phv = phead.tile([HID, A], f32, tag="pmat")
                nc.tensor.matmul(phv[:], vw1h, EMBb[:])
                hv = sp.tile([HID, A], f32, tag="hv")
                hv_cs = sp.tile([U, 1], f32, tag="hv_cs")
                nc.scalar.activation(hv[:], phv[:], Act.Relu,
                                     bias=wp[:, WP_VB1:WP_VB1 + 1],
                                     accum_out=hv_cs[:])
                psv = phead.tile([1, 1], f32, tag="pa")
                nc.tensor.matmul(psv[:], hv_cs[:], wp[:, WP_VW2:WP_VW2 + 1])

                ra = sp.tile([1, 1], f32, tag="ra")
                nc.vector.tensor_reduce(ra[:], pa[:], axis=AxX, op=Alu.add)
                Kb = sp.tile([1, 1], f32, tag="Kb")
                nc.vector.tensor_scalar(
                    Kb[:], psv[:], ra[:], kc[:], Alu.subtract, op1=Alu.add
                )
                nc.vector.scalar_tensor_tensor(
                    FIN[:, sl], pa[:], Kb[:], m10[:, sl], Alu.add, Alu.add
                )
                if b == 0:
                    nc.sync.dma_start(out[b, :][None, :], FIN[:, sl])
                else:
                    nc.scalar.dma_start(out[b, :][None, :], FIN[:, sl])

    return nc


_NC_CACHE: dict[bool, bass.Bass] = {}


def _get_nc(fast: bool) -> bass.Bass:
    if fast not in _NC_CACHE:
        nc = _build_fast() if fast else _build_general()
        nc.finalize()
        _NC_CACHE[fast] = nc
    return _NC_CACHE[fast]


def _make_in_maps_fast(inputs: dict) -> list[dict]:
    x = np.asarray(inputs["x"], dtype=np.float32)
    w16 = np.ascontiguousarray(x[:, 2:A + 2, :].astype(np.float16))
    cur16 = np.ascontiguousarray(x[:, 0, :].astype(np.float16))
    msk16 = np.ascontiguousarray(x[:, 1, :].astype(np.float16))
    wpk = _pack_fast_weights(
        {k: np.asarray(inputs[k], dtype=np.float32) for k in WEIGHT_NAMES}
    )
    in_maps = []
    for c in range(NCORES):
        s = slice(c * BPC, (c + 1) * BPC)
        in_maps.append({
            "w16": np.ascontiguousarray(w16[s]),
            "aux_c": cur16[s].reshape(1, BPC * A),
            "aux_m": msk16[s].reshape(1, BPC * A),
            "wpk": wpk,
        })
    return in_maps


def _make_in_maps_general(inputs: dict) -> list[dict]:
    x = np.ascontiguousarray(np.asarray(inputs["x"], dtype=np.float32))
    wpk = _pack_weights(
        {k: np.asarray(inputs[k], dtype=np.float32) for k in WEIGHT_NAMES}
    )
    in_maps = []
    for c in range(NCORES):
        in_maps.append({
            "xs": np.ascontiguousarray(x[c * BPC: (c + 1) * BPC]),
            "wpack": wpk,
        })
    return in_maps


def run(inputs: dict, trace: bool = False, tmpdir: str | None = None):
    """Returns (output [B, A] f32, BassKernelResults)."""
    x = np.asarray(inputs["x"])
    fast = bool((x[:, 2:, :] != 0.0).all())
    nc = _get_nc(fast)
    in_maps = _make_in_maps_fast(inputs) if fast \
        else _make_in_maps_general(inputs)
    res = run_bass_kernel_spmd(
        nc, in_maps, list(range(NCORES)), trace=trace, tmpdir=tmpdir,
    )
    out = np.concatenate(
        [res.results[i]["out"].reshape(BPC, A) for i in range(NCORES)], axis=0
    )
    return out, res


def kernel(**inputs) -> np.ndarray:
    out, _ = run(inputs)
    return out


# revision 23
# speedup vs baseline: 1.1658x; 1.0192x over previous
"""Trainium2 Bass kernel for nn_DQN_9904194584789 (GNN message passing DQN).

Reference math (B=16, A=256, U=64, T=3):
    cur_sol = x[:,0,:]; mask = x[:,1,:]; w = x[:,2:,:]          # [B,A,A]
    adj = (w != 0)
    e1 = cur_sol[:,:,None] @ W0                                  # rank-1
    e3 = (sum_j relu(w[...,None] * W3) / A) @ W2                 # collapses:
         sum_j relu(w*c) = P*relu(c) + N*relu(-c) elementwise in c, with
         P = sum_j relu(w), N = sum_j relu(-w) = -M, M = sum_j min(w, 0)
         => e3 = P x (relu(W3)@W2/A) + M x (-relu(-W3)@W2/A) = P x Fp + M x Fn
    base = e1 + e3 (loop invariant);  emb_1 = relu(base)
    emb_{t+1} = relu(base + c_t) with c_t = W1^T @ colsum(emb_t)/A   (fast
    path: w has no exact zeros so adj is all-ones and adj@emb collapses to
    a colsum broadcast).
    heads: dueling MLP on emb_3; out = pa + [sum(psv) - sum(pa) + kc] + 10*mask
    with kc = A*v_b2 - (A-1)*a_b3.

Sharding: pure data-parallel over batch B (2 batches per core x 8 cores),
replicated weights host-packed into one [128, NW] buffer (fp16 payloads as
raw bits in f32 columns, incl. a host-packed fp16 identity).  Host work is
only slicing / layout packing / dtype casts (x cast to fp16); all model math
runs on device.

Fast-path structure (both batches fused into [., 512] ops):
  - P/M row-sum stats via tensor_scalar relu/min chunk ops with accum_out
    (DVE 5 chunks + Pool 3 chunks in parallel), cast once to fp16, PE
    transposes -> G [2, 512]
  - base = Fts^T @ G + W0row^T @ cur_row (two matmuls into one PSUM bank)
  - 3 relu iterations on DVE reading base straight from PSUM, with colsum
    accum_out; c = W1^T cs/A via fp16 hi+lo matmuls
  - dueling heads: row-sum corrections come free from accum_out on the
    h2/hv relu legs + two tiny matmuls (no big reduces); 10*mask is a
    rank-1 fp16 matmul accumulated into the pa PSUM bank; kc folds into K.

General path (any exact zero in w): the original f32 kernel with real
adjacency matmuls, unchanged.
"""

import numpy as np
from contextlib import ExitStack

import concourse.bass as bass
import concourse.bacc as bacc
import concourse.tile as tile
from concourse import mybir
from concourse.bass_utils import run_bass_kernel_spmd
from concourse.masks import make_identity

f32 = mybir.dt.float32
f16 = mybir.dt.float16
Alu = mybir.AluOpType
Act = mybir.ActivationFunctionType
AxX = mybir.AxisListType.X

B, A, U, HID = 16, 256, 64, 64
NCORES = 8
BPC = B // NCORES  # batches per core
INV_A = 1.0 / A

WEIGHT_NAMES = [
    "W0", "W1", "W2", "W3",
    "a_w1", "a_b1", "a_w2", "a_b2", "a_w3", "a_b3",
    "v_w1", "v_b1", "v_w2", "v_b2",
]

# ---------------------------------------------------------------------------
# fast-path wpk layout: [128, NW] f32; fp16 payloads packed pairwise as bits.
_c = 0
def _adv(n):
    global _c
    r = _c
    _c += n
    return r

CW1H = _adv(32)     # (W1/A) fp16 hi      [64, 64]
CW1L = _adv(32)     # (W1/A) fp16 lo      [64, 64]
CW2H = _adv(32)     # W2 fp16             [64, 64]
CW3H = _adv(1)      # W3 col fp16         [64, 1]
CAW1 = _adv(32)     # a_w1 fp16           [64, 64]
CAW2 = _adv(16)     # a_w2 fp16           [64, 32]
CVW1 = _adv(32)     # v_w1 fp16           [64, 64]
CW0C = _adv(1)      # W0 col fp16         [64, 1]
CA3M = _adv(1)      # [a_w3; 10.0] fp16   [33, 1]
CVW2H = _adv(1)     # v_w2 fp16 col       [64, 1]
CA3NH = _adv(1)     # -a_w3 fp16 col      [32, 1]
CAB1 = _adv(1)      # a_b1 f32 col        [64, 1]
CVB1 = _adv(1)      # v_b1 f32 col        [64, 1]
CAB2 = _adv(1)      # a_b2 f32 col        [32, 1]
CAB3 = _adv(1)      # a_b3 scalar         [1, 1]
CVB2 = _adv(1)      # v_b2 scalar         [1, 1]
CIDT = _adv(64)     # identity fp16       [128, 128]
NW = _c


def _pack_fast_weights(inputs: dict) -> np.ndarray:
    wp = np.zeros((128, NW), dtype=np.float32)

    def place(col, arr):  # fp16 bits packed pairwise into f32 columns
        raw = np.ascontiguousarray(
            np.asarray(arr, np.float32).astype(np.float16)
        ).view(np.uint16)
        k = raw.shape[1]
        pad = np.zeros((raw.shape[0], (k + 1) // 2 * 2), np.uint16)
        pad[:, :k] = raw
        fview = pad.view(np.float32)
        wp[:fview.shape[0], col:col + fview.shape[1]] = fview

    # 1/A = 1/256 is a power of two: W1/A is an exact f32 re-encoding;
    # hi+lo fp16 split keeps the W1 product near-f32 accurate.
    w1a = np.asarray(inputs["W1"], np.float32) * INV_A
    w1ah = w1a.astype(np.float16).astype(np.float32)
    place(CW1H, w1a)
    place(CW1L, w1a - w1ah)
    place(CW2H, inputs["W2"])
    place(CW3H, np.asarray(inputs["W3"], np.float32).T)   # [64, 1]
    place(CAW1, inputs["a_w1"])
    place(CAW2, inputs["a_w2"])
    place(CVW1, inputs["v_w1"])
    place(CW0C, np.asarray(inputs["W0"], np.float32).T)  # [64, 1] col
    a3m = np.zeros((33, 1), np.float32)
    a3m[:32, 0] = np.asarray(inputs["a_w3"], np.float32)[:, 0]
    a3m[32, 0] = 10.0
    place(CA3M, a3m)
    place(CVW2H, np.asarray(inputs["v_w2"], np.float32))
    place(CA3NH, -np.asarray(inputs["a_w3"], np.float32))
    place(CIDT, np.eye(128, dtype=np.float32))
    wp[:64, CAB1] = inputs["a_b1"]
    wp[:64, CVB1] = inputs["v_b1"]
    wp[:32, CAB2] = inputs["a_b2"]
    wp[0, CAB3] = inputs["a_b3"][0]
    wp[0, CVB2] = inputs["v_b2"][0]
    return wp


def _build_fast() -> bass.Bass:
    # Bacc (not raw Bass): its finalize() runs move_matmul_waits_to_ldweights
    # + generate_event_semaphores for the TRN2 one-wait-per-inst constraint.
    nc = bacc.Bacc(
        "TRN2", target_bir_lowering=False, debug=False, num_devices=NCORES
    )
    w16d = nc.declare_dram_parameter("w16", [BPC, A, A], f16, isOutput=False)
    auxc = nc.declare_dram_parameter("aux_c", [1, BPC * A], f16, isOutput=False)
    auxm = nc.declare_dram_parameter("aux_m", [1, BPC * A], f16, isOutput=False)
    wpd = nc.declare_dram_parameter("wpk", [128, NW], f32, isOutput=False)
    outd = nc.declare_dram_parameter("out", [1, BPC * A], f32, isOutput=True)

    NBA = BPC * A  # 512

    with tile.TileContext(nc) as tc, ExitStack() as ctx, \
         nc.allow_low_precision("fp16 colsum accums; tol 5e-3"):
        cp = ctx.enter_context(tc.tile_pool(name="const", bufs=1))
        sp = ctx.enter_context(tc.tile_pool(name="scratch", bufs=2))
        dv = ctx.enter_context(tc.tile_pool(name="dumpv", bufs=2))
        dp = ctx.enter_context(tc.tile_pool(name="dumpp", bufs=2))

        # ---------- input DMAs, issued first on distinct queues -----------
        # batch1 feeds the (slower) ACT chain -> land it first on sync.
        wt4 = cp.tile([128, 2 * BPC, A], f16, tag="wt4")
        nc.sync.dma_start(
            wt4[:, 2:4, :],
            w16d[1, :, :].rearrange("(t p) j -> p t j", p=128),
        )
        wp = cp.tile([128, NW], f32, tag="wp")
        nc.scalar.dma_start(wp[:], wpd[:])
        nc.scalar.dma_start(
            wt4[:, 0:2, :],
            w16d[0, :, :].rearrange("(t p) j -> p t j", p=128),
        )
        # cur_sol rides directly into partition 2 of each per-batch G so
        # base = [Ft; Fs; W0]^T @ [T; S; cur] is a single K=3 matmul.
        GB = cp.tile([3, 2, 128], f16, tag="GB")
        nc.gpsimd.dma_start(GB[2:3, :, :].rearrange("r c p -> r (c p)"),
                            auxc[:, A:2 * A])
        GA = cp.tile([3, 2, 128], f16, tag="GA")
        nc.gpsimd.dma_start(GA[2:3, :, :].rearrange("r c p -> r (c p)"),
                            auxc[:, 0:A])
        # per-batch h2 tiles [33, 256]: partition 32 carries the mask row so
        # the pa matmul computes out_a + 10*mask in one [33,1]^T @ [33,256].
        h2A = cp.tile([HID // 2 + 1, A], f16, tag="h2A")
        nc.gpsimd.dma_start(h2A[32:33, :], auxm[:, 0:A])
        h2B = cp.tile([HID // 2 + 1, A], f16, tag="h2B")
        nc.gpsimd.dma_start(h2B[32:33, :], auxm[:, A:2 * A])

        # views of host-packed weights
        w1hh = wp[0:64, CW1H:CW1H + 32].bitcast(f16)          # [64, 64]
        w1lh = wp[0:64, CW1L:CW1L + 32].bitcast(f16)          # [64, 64]
        w2h = wp[0:64, CW2H:CW2H + 32].bitcast(f16)           # [64, 64]
        w3h = wp[0:64, CW3H:CW3H + 1].bitcast(f16)[:, 0:1]    # [64, 1]
        aw1h = wp[0:64, CAW1:CAW1 + 32].bitcast(f16)          # [64, 64]
        aw2h = wp[0:64, CAW2:CAW2 + 16].bitcast(f16)          # [64, 32]
        vw1h = wp[0:64, CVW1:CVW1 + 32].bitcast(f16)          # [64, 64]
        w0c = wp[0:64, CW0C:CW0C + 1].bitcast(f16)[:, 0:1]    # [64, 1]
        a3m = wp[0:33, CA3M:CA3M + 1].bitcast(f16)[:, 0:1]    # [33, 1]
        vw2h = wp[0:64, CVW2H:CVW2H + 1].bitcast(f16)[:, 0:1]
        a3nh = wp[0:32, CA3NH:CA3NH + 1].bitcast(f16)[:, 0:1]
        ident = wp[:, CIDT:CIDT + 64].bitcast(f16)            # [128, 128]
        ab1f = wp[0:64, CAB1:CAB1 + 1]
        vb1f = wp[0:64, CVB1:CVB1 + 1]
        ab2f = wp[0:32, CAB2:CAB2 + 1]

        with tc.tile_pool(name="pmm", bufs=2, space="PSUM") as pmm, \
             tc.tile_pool(name="pbase", bufs=1, space="PSUM") as pbase, \
             tc.tile_pool(name="phead", bufs=1, space="PSUM") as phead:

            # ---------- setup (runs in the input-DMA shadow) --------------
            # e3 = T x (|W3|@W2/2A) + S x (W3@W2/2A) with T = sum|w|,
            # S = sum w  (from e3 = P*Fp + M*Fn, P=(S+T)/2, M=(S-T)/2).
            w3p = cp.tile([U, 1], f16, tag="w3p")
            nc.scalar.activation(w3p[:], w3h, Act.Abs, scale=0.5 * INV_A)
            w3i = cp.tile([U, 1], f16, tag="w3i")
            nc.scalar.activation(w3i[:], w3h, Act.Identity, scale=0.5 * INV_A)
            psF2 = pmm.tile([U, 2], f32, tag="tp")
            nc.tensor.matmul(psF2[:, 0:1], w2h, w3p[:])
            nc.tensor.matmul(psF2[:, 1:2], w2h, w3i[:])
            Fc16 = cp.tile([U, 3], f16, tag="Fc16")
            nc.scalar.activation(Fc16[:, 0:2], psF2[:], Act.Identity)
            nc.scalar.activation(Fc16[:, 2:3], w0c, Act.Identity)
            psFT = pmm.tile([3, U], f16, tag="tp")
            nc.tensor.transpose(psFT[:], Fc16[:], ident[0:U, 0:U])
            Fts = cp.tile([3, U], f16, tag="Fts")
            nc.scalar.activation(Fts[:], psFT[:], Act.Identity)

            # kc = A*v_b2 - (A-1)*a_b3 (scalar part of the dueling combine)
            t256 = cp.tile([1, 1], f32, tag="t256")
            nc.gpsimd.tensor_scalar(
                t256[:], wp[0:1, CVB2:CVB2 + 1], float(A), None, Alu.mult
            )
            kc = cp.tile([1, 1], f32, tag="kc")
            nc.gpsimd.tensor_scalar(
                kc[:], wp[0:1, CAB3:CAB3 + 1], -float(A - 1), t256[:],
                Alu.mult, Alu.add,
            )
            zz = cp.tile([U, A], f16, tag="zz")
            nc.gpsimd.memset(zz[:], 0.0)

            # ---------- T/S stats -> G -> base, per batch (B first) -------
            # batch1's data lands first and feeds the slower ACT chain.
            TScB = cp.tile([128, 4], f16, tag="TScB")
            with nc.allow_low_precision("fp16 row-sum stats; tol 5e-3"):
                nc.vector.tensor_reduce(
                    TScB[:, 0:4:2], wt4[:, 2:4, :], axis=AxX,
                    op=Alu.add, apply_absolute_value=True,
                )
                nc.vector.tensor_reduce(
                    TScB[:, 1:4:2], wt4[:, 2:4, :], axis=AxX,
                    op=Alu.add
                )
            psT = pmm.tile([2, 2, 128], f16, tag="tp")
            nc.tensor.transpose(psT[:, 0, :], TScB[:, 0:2], ident[:])
            nc.tensor.transpose(psT[:, 1, :], TScB[:, 2:4], ident[:])
            nc.vector.tensor_copy(GB[0:2, :, :], psT[:])
            ps_bB = pbase.tile([U, A], f32, tag="psbB")
            nc.tensor.matmul(ps_bB[:], Fts[:],
                             GB[:].rearrange("r c p -> r (c p)"))

            TScA = cp.tile([128, 4], f16, tag="TScA")
            with nc.allow_low_precision("fp16 row-sum stats; tol 5e-3"):
                nc.vector.tensor_reduce(
                    TScA[:, 0:4:2], wt4[:, 0:2, :], axis=AxX,
                    op=Alu.add, apply_absolute_value=True,
                )
                nc.vector.tensor_reduce(
                    TScA[:, 1:4:2], wt4[:, 0:2, :], axis=AxX,
                    op=Alu.add
                )
            psT = pmm.tile([2, 2, 128], f16, tag="tp")
            nc.tensor.transpose(psT[:, 0, :], TScA[:, 0:2], ident[:])
            nc.tensor.transpose(psT[:, 1, :], TScA[:, 2:4], ident[:])
            nc.vector.tensor_copy(GA[0:2, :, :], psT[:])
            ps_bA = pbase.tile([U, A], f32, tag="psbA")
            nc.tensor.matmul(ps_bA[:], Fts[:],
                             GA[:].rearrange("r c p -> r (c p)"))

            # ---------- message passing ----------------------------------
            # independent per-batch chains: batch0 on DVE, batch1 on ACT;
            # each chain is engine-self-contained (PSUM banks and scratch
            # tiles are single-engine to avoid serializing hazards).
            cs_a = sp.tile([U, 1], f16, tag="cs_a")
            cs_b = sp.tile([U, 1], f16, tag="cs_b")
            dm = dp.tile([U, A], f16, tag="dms")
            nc.scalar.activation(dm[:], ps_bB[:], Act.Relu,
                                 accum_out=cs_b[:])
            dm = dv.tile([U, A], f16, tag="dme")
            nc.vector.tensor_scalar(
                dm[:], ps_bA[:], 0.0, None, Alu.max,
                op1=Alu.add, accum_out=cs_a[:],
            )
            emb_a = cp.tile([U, A], f16, tag="emb_a")
            emb_b = cp.tile([U, A], f16, tag="emb_b")
            for it in range(2):
                psCB = pmm.tile([U, 1], f32, tag="tp")
                nc.tensor.matmul(psCB[:], w1hh, cs_b[:])
                csbB = sp.tile([U, 1], f32, tag="csbB")
                nc.scalar.activation(csbB[:], psCB[:], Act.Identity)
                psCA = pmm.tile([U, 1], f32, tag="tp")
                nc.tensor.matmul(psCA[:], w1hh, cs_a[:])
                csbA = sp.tile([U, 1], f32, tag="csbA")
                nc.vector.tensor_copy(csbA[:], psCA[:])
                if it == 0:
                    cs_a = sp.tile([U, 1], f16, tag="cs_a2")
                    cs_b = sp.tile([U, 1], f16, tag="cs_b2")
                    dm = dp.tile([U, A], f16, tag="dms")
                    nc.scalar.activation(dm[:], ps_bB[:], Act.Relu,
                                         bias=csbB[:], accum_out=cs_b[:])
                    dm = dv.tile([U, A], f16, tag="dme")
                    nc.vector.scalar_tensor_tensor(
                        dm[:], ps_bA[:], csbA[:], zz[:],
                        Alu.add, Alu.max, accum_out=cs_a[:],
                    )
                else:
                    nc.scalar.activation(emb_b[:], ps_bB[:],
                                         Act.Relu, bias=csbB[:])
                    nc.vector.tensor_scalar(
                        emb_a[:], ps_bA[:], csbA[:], 0.0,
                        Alu.add, op1=Alu.max,
                    )

            # ---------- dueling heads: per-batch chains (B first) ---------
            ph1B = phead.tile([HID, A], f32, tag="hb")
            nc.tensor.matmul(ph1B[:], aw1h, emb_b[:])
            phvB = phead.tile([HID, A], f32, tag="vb")
            nc.tensor.matmul(phvB[:], vw1h, emb_b[:])
            h1B = cp.tile([HID, A], f16, tag="h1B")
            nc.scalar.activation(h1B[:], ph1B[:], Act.Relu, bias=ab1f)

            ph1A = phead.tile([HID, A], f32, tag="ha")
            nc.tensor.matmul(ph1A[:], aw1h, emb_a[:])
            phvA = phead.tile([HID, A], f32, tag="va")
            nc.tensor.matmul(phvA[:], vw1h, emb_a[:])
            h1A = cp.tile([HID, A], f16, tag="h1A")
            nc.vector.tensor_scalar(
                h1A[:], ph1A[:], ab1f, 0.0, Alu.add, op1=Alu.max,
            )

            ph2B = phead.tile([HID // 2, A], f32, tag="hb")
            nc.tensor.matmul(ph2B[:], aw2h, h1B[:])
            hvcsb = sp.tile([U, 1], f16, tag="hvcsb")
            dmv = dp.tile([U, A], f16, tag="dms")
            nc.scalar.activation(dmv[:], phvB[:], Act.Relu, bias=vb1f,
                                 accum_out=hvcsb[:])
            h2csb = sp.tile([HID // 2, 1], f16, tag="h2csb")
            nc.scalar.activation(h2B[0:32, :], ph2B[:], Act.Relu,
                                 bias=ab2f, accum_out=h2csb[:])

            ph2A = phead.tile([HID // 2, A], f32, tag="ha")
            nc.tensor.matmul(ph2A[:], aw2h, h1A[:])
            hvcsa = sp.tile([U, 1], f16, tag="hvcsa")
            dmv = dv.tile([U, A], f16, tag="dme")
            nc.vector.scalar_tensor_tensor(
                dmv[:], phvA[:], vb1f, zz[:], Alu.add, Alu.max,
                accum_out=hvcsa[:],
            )
            h2csa = sp.tile([HID // 2, 1], f16, tag="h2csa")
            nc.vector.scalar_tensor_tensor(
                h2A[0:32, :], ph2A[:], ab2f, zz[0:32, :],
                Alu.add, Alu.max, accum_out=h2csa[:],
            )

            # K columns: fp16 casts of the accums feed tiny PE matmuls
            psK1 = pmm.tile([1, 1], f32, tag="tp")
            nc.tensor.matmul(psK1[:], vw2h, hvcsb[:], start=True, stop=False)
            nc.tensor.matmul(psK1[:], a3nh, h2csb[:], start=False, stop=True)
            paB = phead.tile([1, A], f32, tag="hb")
            nc.tensor.matmul(paB[:], a3m, h2B[:])
            K1 = sp.tile([1, 1], f32, tag="K1")
            nc.vector.tensor_scalar(K1[:], psK1[:], kc[0:1, 0:1], None,
                                    Alu.add)
            FINb = cp.tile([1, A], f32, tag="FINb")
            nc.scalar.activation(FINb[:], paB[:], Act.Identity,
                                 bias=K1[0:1, 0:1])
            nc.scalar.dma_start(outd[:, A:2 * A], FINb[:])

            psK0 = pmm.tile([1, 1], f32, tag="tp")
            nc.tensor.matmul(psK0[:], vw2h, hvcsa[:], start=True, stop=False)
            nc.tensor.matmul(psK0[:], a3nh, h2csa[:], start=False, stop=True)
            paA = phead.tile([1, A], f32, tag="ha")
            nc.tensor.matmul(paA[:], a3m, h2A[:])
            K0 = sp.tile([1, 1], f32, tag="K0")
            nc.vector.tensor_scalar(K0[:], psK0[:], kc[0:1, 0:1], None,
                                    Alu.add)
            FINa = cp.tile([1, A], f32, tag="FINa")
            nc.vector.tensor_scalar(
                FINa[:], paA[:], K0[0:1, 0:1], None, Alu.add
            )
            nc.sync.dma_start(outd[:, 0:A], FINa[:])

    return nc


# ---------------------------------------------------------------------------
# General path (exact zeros in w): original f32 kernel, unchanged.

WP_W1 = 0
WP_W2 = 64
WP_W3 = 128
WP_AB1 = 129
WP_VB1 = 130
WP_AB2 = 131
WP_VW2 = 132
WP_AB3 = 133
WP_VB2 = 134
WP_AW1H = 135
WP_AW2H = 167
WP_AW3H = 183
WP_VW1H = 184
WP_W0C = 216
WP_W1HH = 217
WP_W1LH = 249
WP_AW3F = 280
WP_W2H = 281
NWP = 313


def _pack_weights(inputs: dict) -> np.ndarray:
    wp = np.zeros((64, NWP), dtype=np.float32)
    wp[:, WP_W1:WP_W1 + 64] = inputs["W1"]
    wp[:, WP_W2:WP_W2 + 64] = inputs["W2"]
    wp[:, WP_W3] = inputs["W3"][0]
    wp[:, WP_AB1] = inputs["a_b1"]
    wp[:, WP_VB1] = inputs["v_b1"]
    wp[:32, WP_AB2] = inputs["a_b2"]
    wp[:, WP_VW2] = inputs["v_w2"][:, 0]
    wp[0, WP_AB3] = inputs["a_b3"][0]
    wp[0, WP_VB2] = inputs["v_b2"][0]
    wp[:32, WP_AW3F] = inputs["a_w3"][:, 0]

    def place(col, arr):
        raw = np.ascontiguousarray(
            np.asarray(arr, np.float32).astype(np.float16)
        ).view(np.uint16)
        k = raw.shape[1]
        pad = np.zeros((raw.shape[0], (k + 1) // 2 * 2), np.uint16)
        pad[:, :k] = raw
        fview = pad.view(np.float32)
        wp[:fview.shape[0], col:col + fview.shape[1]] = fview

    place(WP_AW1H, inputs["a_w1"])
    place(WP_AW2H, inputs["a_w2"])
    place(WP_AW3H, inputs["a_w3"][:, 0:1])
    place(WP_VW1H, inputs["v_w1"])
    place(WP_W0C, inputs["W0"].T)
    place(WP_W2H, inputs["W2"])
    w1 = np.asarray(inputs["W1"], np.float32)
    w1h = w1.astype(np.float16).astype(np.float32)
    place(WP_W1HH, w1)
    place(WP_W1LH, w1 - w1h)
    return wp


def _build_general() -> bass.Bass:
    nc = bacc.Bacc(
        "TRN2", target_bir_lowering=False, debug=False, num_devices=NCORES
    )
    xs = nc.declare_dram_parameter("xs", [BPC, A + 2, A], f32, isOutput=False)
    wpd = nc.declare_dram_parameter("wpack", [64, NWP], f32, isOutput=False)
    out = nc.declare_dram_parameter("out", [BPC, A], f32, isOutput=True)

    with tile.TileContext(nc) as tc, ExitStack() as ctx, \
         nc.allow_low_precision("fp16 colsum accums; tol 5e-3"):
        cp = ctx.enter_context(tc.tile_pool(name="const", bufs=1))
        sp = ctx.enter_context(tc.tile_pool(name="scratch", bufs=2))

        wp = cp.tile([64, NWP], f32, tag="wp")
        nc.sync.dma_start(wp[:], wpd[:])
        wt4 = cp.tile([128, 2 * BPC, A], f32, tag="wt4")
        for b in range(BPC):
            nc.scalar.dma_start(
                wt4[:, 2 * b: 2 * b + 2, :],
                xs[b, 2: A + 2, :].rearrange("(t p) j -> p t j", p=128),
            )
        csc = cp.tile([128, 2 * BPC], f32, tag="csc")
        for b in range(BPC):
            nc.gpsimd.dma_start(
                csc[:, 2 * b: 2 * b + 2],
                xs[b, 0, :].rearrange("(t p) -> p t", p=128),
            )
        mrow = cp.tile([1, BPC * A], f32, tag="mrow")
        nc.gpsimd.dma_start(
            mrow[:].rearrange("p (b a) -> p b a", b=BPC),
            xs[:, 1, :][None, :, :],
        )

        aw1h = wp[:, WP_AW1H:WP_AW1H + 32].bitcast(f16)
        aw2h = wp[:, WP_AW2H:WP_AW2H + 16].bitcast(f16)
        aw3h = wp[0:32, WP_AW3H:WP_AW3H + 1].bitcast(f16)[:, 0:1]
        vw1h = wp[:, WP_VW1H:WP_VW1H + 32].bitcast(f16)
        w0c = wp[:, WP_W0C:WP_W0C + 1].bitcast(f16)[:, 0:1]

        ident = cp.tile([128, 128], f16, tag="ident")
        make_identity(nc, ident[:])
        identf = cp.tile([128, 128], f32, tag="identf")
        make_identity(nc, identf[:])

        with tc.tile_pool(name="psetup", bufs=2, space="PSUM") as psetup:
            w2h = wp[:, WP_W2H:WP_W2H + 32].bitcast(f16)
            w3p = cp.tile([U, 1], f16, tag="w3p")
            nc.scalar.activation(w3p[:], wp[:, WP_W3:WP_W3 + 1], Act.Relu)
            w3n = cp.tile([U, 1], f16, tag="w3n")
            nc.scalar.activation(w3n[:], wp[:, WP_W3:WP_W3 + 1], Act.Relu,
                                 scale=-1.0)
            Fc = cp.tile([U, 3], f16, tag="Fc")
            nc.vector.tensor_copy(Fc[:, 0:1], w0c)
            pspc = psetup.tile([U, 1], f32, tag="pscol")
            nc.tensor.matmul(pspc[:], w2h, w3p[:])
            nc.scalar.mul(Fc[:, 1:2], pspc[:], INV_A)
            psnc = psetup.tile([U, 1], f32, tag="pscol")
            nc.tensor.matmul(psnc[:], w2h, w3n[:])
            nc.scalar.mul(Fc[:, 2:3], psnc[:], INV_A)
            psF = psetup.tile([3, U], f16, tag="psF")
            nc.tensor.transpose(psF[:], Fc[:], ident[0:U, 0:U])
            F = cp.tile([3, U], f16, tag="F")
            nc.vector.tensor_copy(F[:], psF[:])

        t256 = cp.tile([1, 1], f32, tag="t256")
        nc.gpsimd.tensor_scalar(
            t256[:], wp[0:1, WP_VB2:WP_VB2 + 1], float(A), None, Alu.mult
        )
        kc = cp.tile([1, 1], f32, tag="kc")
        nc.gpsimd.tensor_scalar(
            kc[:], wp[0:1, WP_AB3:WP_AB3 + 1], -float(A - 1), t256[:],
            Alu.mult, Alu.add,
        )

        m10 = cp.tile([1, BPC * A], f32, tag="m10")
        nc.scalar.mul(m10[:], mrow[:], 10.0)

        FIN = cp.tile([1, BPC * A], f32, tag="FIN")

        with tc.tile_pool(name="pmm", bufs=1, space="PSUM") as pmm, \
             tc.tile_pool(name="pbase", bufs=2, space="PSUM") as pbase, \
             tc.tile_pool(name="phead", bufs=2, space="PSUM") as phead:
            for b in range(BPC):
                Tb = sp.tile([128, 2], f32, tag="Tb")
                nc.vector.tensor_reduce(
                    Tb[:], wt4[:, 2 * b: 2 * b + 2, :], axis=AxX, op=Alu.add,
                    apply_absolute_value=True,
                )
                Sb = sp.tile([128, 2], f32, tag="Sb")
                nc.vector.tensor_reduce(
                    Sb[:], wt4[:, 2 * b: 2 * b + 2, :], axis=AxX, op=Alu.add
                )
                Sh = sp.tile([128, 2], f32, tag="Sh")
                nc.gpsimd.tensor_scalar(Sh[:], Sb[:], 0.5, None, Alu.mult)

                G = sp.tile([3, A], f16, tag="G")
                for t in range(2):
                    Cc = sp.tile([128, 3], f16, tag="Cc")
                    nc.gpsimd.tensor_copy(
                        Cc[:, 0:1], csc[:, 2 * b + t: 2 * b + t + 1]
                    )
                    nc.vector.scalar_tensor_tensor(
                        Cc[:, 1:2], Tb[:, t: t + 1], 0.5, Sh[:, t: t + 1],
                        Alu.mult, Alu.add,
                    )
                    nc.vector.scalar_tensor_tensor(
                        Cc[:, 2:3], Tb[:, t: t + 1], 0.5, Sh[:, t: t + 1],
                        Alu.mult, Alu.subtract,
                    )
                    tpc = pmm.tile([3, 128], f16, tag="tp1")
                    nc.tensor.transpose(tpc[:], Cc[:], ident[:])
                    nc.vector.tensor_copy(
                        G[:, t * 128: (t + 1) * 128], tpc[:]
                    )

                ps_base = pbase.tile([U, A], f32, tag="psbase")
                nc.tensor.matmul(ps_base[:], F[:], G[:])

                wt = wt4[:, 2 * b: 2 * b + 2, :]
                adjT = sp.tile([128, 2, A], f32, tag="adjT")
                for at in range(2):
                    for jt in range(2):
                        ptr = pmm.tile([128, 128], f32, tag="tp1")
                        nc.tensor.transpose(
                            ptr[:], wt[:, at, jt * 128: (jt + 1) * 128],
                            identf[:],
                        )
                        nc.vector.tensor_scalar(
                            adjT[:, jt, at * 128: (at + 1) * 128],
                            ptr[:], 0.0, None, Alu.not_equal,
                        )
                embT = sp.tile([U, A], f32, tag="embT")
                nc.vector.tensor_scalar(
                    embT[:], ps_base[:], 0.0, None, Alu.max
                )
                EMBb = None
                for it in range(2):
                    nat = sp.tile([128, 2, U], f32, tag="nat")
                    for ht in range(2):
                        pnat = pmm.tile([128, U], f32, tag="tp1")
                        nc.tensor.transpose(
                            pnat[:], embT[:, ht * 128: (ht + 1) * 128],
                            identf[0:U, 0:U],
                        )
                        nc.vector.tensor_copy(nat[:, ht, :], pnat[:])
                    ps_y = pmm.tile([U, A], f32, tag="tp1")
                    nc.tensor.matmul(ps_y[:], nat[:, 0, :], adjT[:, 0, :],
                                     start=True, stop=False)
                    nc.tensor.matmul(ps_y[:], nat[:, 1, :], adjT[:, 1, :],
                                     start=False, stop=True)
                    ysb = sp.tile([U, A], f32, tag="ysb")
                    nc.vector.tensor_scalar(ysb[:], ps_y[:], INV_A, None,
                                            Alu.mult)
                    ps_it = pbase.tile([U, A], f32, tag="psbase")
                    nc.tensor.matmul(ps_it[:], F[:], G[:],
                                     start=True, stop=False)
                    nc.tensor.matmul(ps_it[:], wp[:, WP_W1:WP_W1 + 64],
                                     ysb[:], start=False, stop=True)
                    if it == 0:
                        embT = sp.tile([U, A], f32, tag="embT")
                        nc.vector.tensor_scalar(
                            embT[:], ps_it[:], 0.0, None, Alu.max
                        )
                    else:
                        EMBb = sp.tile([U, A], f16, tag="EMBb")
                        nc.vector.tensor_scalar(
                            EMBb[:], ps_it[:], 0.0, None, Alu.max
                        )

                sl = slice(b * A, (b + 1) * A)
                ph1 = phead.tile([HID, A], f32, tag="pmat")
                nc.tensor.matmul(ph1[:], aw1h, EMBb[:])
                h1 = sp.tile([HID, A], f16, tag="h1")
                nc.scalar.activation(h1[:], ph1[:], Act.Relu,
                                     bias=wp[:, WP_AB1:WP_AB1 + 1])
                ph2 = phead.tile([HID // 2, A], f32, tag="pmat")
                nc.tensor.matmul(ph2[:], aw2h, h1[:])
                h2 = sp.tile([HID // 2, A], f16, tag="h2")
                nc.vector.tensor_scalar(
                    h2[:], ph2[:], wp[0:32, WP_AB2:WP_AB2 + 1], 0.0,
                    Alu.add, op1=Alu.max,
                )
                pa = phead.tile([1, A], f32, tag="pa")
                nc.tensor.matmul(pa[:], aw3h, h2[:])

                phv = phead.tile([HID, A], f32, tag="pmat")
                nc.tensor.matmul(phv[:], vw1h, EMBb[:])
                hv = sp.tile([HID, A], f32, tag="hv")
                hv_cs = sp.tile([U, 1], f32, tag="hv_cs")
                nc.scalar.activation(hv[:], phv[:], Act.Relu,
                                     bias=wp[:, WP_VB1:WP_VB1 + 1],
                                     accum_out=hv_cs[:])
                psv = phead.tile([1, 1], f32, tag="pa")
                nc.tensor.matmul(psv[:], hv_cs[:], wp[:, WP_VW2:WP_VW2 + 1])

                ra = sp.tile([1, 1], f32, tag="ra")
                nc.vector.tensor_reduce(ra[:], pa[:], axis=AxX, op=Alu.add)
                Kb = sp.tile([1, 1], f32, tag="Kb")
                nc.vector.tensor_scalar(
                    Kb[:], psv[:], ra[:], kc[:], Alu.subtract, op1=Alu.add
                )
                nc.vector.scalar_tensor_tensor(
                    FIN[:, sl], pa[:], Kb[:], m10[:, sl], Alu.add, Alu.add
                )
                if b == 0:
                    nc.sync.dma_start(out[b, :][None, :], FIN[:, sl])
                else:
                    nc.scalar.dma_start(out[b, :][None, :], FIN[:, sl])

    return nc


_NC_CACHE: dict[bool, bass.Bass] = {}


def _get_nc(fast: bool) -> bass.Bass:
    if fast not in _NC_CACHE:
        nc = _build_fast() if fast else _build_general()
        nc.finalize()
        _NC_CACHE[fast] = nc
    return _NC_CACHE[fast]


def _make_in_maps_fast(inputs: dict) -> list[dict]:
    x = np.asarray(inputs["x"], dtype=np.float32)
    w16 = np.ascontiguousarray(x[:, 2:A + 2, :].astype(np.float16))
    cur16 = np.ascontiguousarray(x[:, 0, :].astype(np.float16))
    msk16 = np.ascontiguousarray(x[:, 1, :].astype(np.float16))
    wpk = _pack_fast_weights(
        {k: np.asarray(inputs[k], dtype=np.float32) for k in WEIGHT_NAMES}
    )
    in_maps = []
    for c in range(NCORES):
        s = slice(c * BPC, (c + 1) * BPC)
        in_maps.append({
            "w16": np.ascontiguousarray(w16[s]),
            "aux_c": cur16[s].reshape(1, BPC * A),
            "aux_m": msk16[s].reshape(1, BPC * A),
            "wpk": wpk,
        })
    return in_maps


def _make_in_maps_general(inputs: dict) -> list[dict]:
    x = np.ascontiguousarray(np.asarray(inputs["x"], dtype=np.float32))
    wpk = _pack_weights(
        {k: np.asarray(inputs[k], dtype=np.float32) for k in WEIGHT_NAMES}
    )
    in_maps = []
    for c in range(NCORES):
        in_maps.append({
            "xs": np.ascontiguousarray(x[c * BPC: (c + 1) * BPC]),
            "wpack": wpk,
        })
    return in_maps


def run(inputs: dict, trace: bool = False, tmpdir: str | None = None):
    """Returns (output [B, A] f32, BassKernelResults)."""
    x = np.asarray(inputs["x"])
    fast = bool((x[:, 2:, :] != 0.0).all())
    nc = _get_nc(fast)
    in_maps = _make_in_maps_fast(inputs) if fast \
        else _make_in_maps_general(inputs)
    res = run_bass_kernel_spmd(
        nc, in_maps, list(range(NCORES)), trace=trace, tmpdir=tmpdir,
    )
    out = np.concatenate(
        [res.results[i]["out"].reshape(BPC, A) for i in range(NCORES)], axis=0
    )
    return out, res


def kernel(**inputs) -> np.ndarray:
    out, _ = run(inputs)
    return out
